# revision 19
# baseline (speedup 1.0000x reference)
"""Trainium2 Bass kernel for nn_BaseGinNetwork (GIN message passing).

Self-contained: host-side sharding prep (numpy) + one SPMD Bass/Tile program
run on 8 NeuronCores via bass_utils.run_bass_kernel_spmd.

Sharding:
- tasks row-sharded NT/8 per core; VM nodes replicated; compat edges
  src-sharded with a dense per-core count matrix M driving the VM aggregation
  as plain matmuls; partial VM aggregates merged through a concat-AllGather.
- dep edges dst-sharded; x[src] fetched by indirect-DMA gather from an
  AllGathered task table; scatter-add done as one-hot matmuls into PSUM.
- MLP chain runs feature-major so BN/bias/relu are per-partition ACT ops.
- edge_embeddings emitted as one-hot PE expansions; halves assembled on host.
"""
import contextlib
import copy as _copy
import dataclasses as _dc
import math
import os
import sys

import numpy as np

sys.path.insert(0, os.path.dirname(os.path.abspath(__file__)))

import concourse.bass as bass
import concourse.mybir as mybir
import concourse.tile as tile
from concourse.bass_utils import run_bass_kernel_spmd
from concourse.masks import make_identity
from concourse.vector_clock import ScopedClock as _ScopedClock

F32 = mybir.dt.float32
I32 = mybir.dt.int32
P = 128
NCORE = 8
HID, EMB = 256, 128

# ---------------------------------------------------------------- walrus fix
# This container's walrus encodes only ONE sync wait per instruction; Tile
# emits multi-wait instructions.  Split extra waits onto standalone wait-only
# EventSemaphore instructions placed just before, on the same engine.


def _patched_drain_and_barrier(self, tick_clock, wait_clock):
    nc = self.nc
    drain_inst = nc.sync.drain()
    wait_clock.add_sem_waits(
        drain_inst.ins, _ScopedClock({None: tick_clock.global_clock})
    )
    si = drain_inst.ins.sync_info
    ow = list(si.on_wait or []) if si is not None else []
    if len(ow) > 1:
        si.on_wait = ow[:1]
        drain_inst.ins.sync_info = si
        for w in ow[1:]:
            extra = nc.sync.drain()
            esi = extra.ins.sync_info
            if esi is None:
                esi = _dc.replace(si, on_wait=[w], on_update=[])
            else:
                esi.on_wait = [w]
                esi.on_update = []
            extra.ins.sync_info = esi
    nc.all_engine_barrier()
    assert self.sems is not None
    popped = nc._tile_sem_poison_stack.pop()
    assert popped is self._sem_poison
    nc.clear_and_free_semaphores(list(self.sems.allocated().values()))
    nc.all_engine_barrier()


def _split_multiwaits(nc):
    template = None
    for bb in nc.main_func.blocks:
        for ins in bb.instructions:
            if type(ins).__name__ == "InstEventSemaphore":
                template = ins
                break
        if template is not None:
            break
    assert template is not None
    counter = 0
    for bb in nc.main_func.blocks:
        insns = bb.instructions
        new_list = []
        for ins in insns:
            si = getattr(ins, "sync_info", None)
            ow = list(si.on_wait) if (si is not None and si.on_wait) else []
            if len(ow) > 1:
                for w in ow[:-1]:
                    ev = _copy.deepcopy(template)
                    ev.name = f"wsplit_{counter}"
                    counter += 1
                    ev.engine = ins.engine
                    esi = ev.sync_info
                    esi.on_wait = [w]
                    esi.on_update = []
                    ev.sync_info = esi
                    new_list.append(ev)
                si.on_wait = [ow[-1]]
                ins.sync_info = si
            new_list.append(ins)
        insns[:] = new_list


def _install_ntff_hook():
    import types

    try:
        from antenv.axon_hooks import get_axon_ntff_profile_hook  # noqa: F401

        return
    except ImportError:
        pass
    try:
        import antenv
        from trn_agent_boot.trn_boot import _ntff_profile_via_ctypes
    except ImportError:
        return
    mod = types.ModuleType("antenv.axon_hooks")
    state = {"hook": _ntff_profile_via_ctypes("/opt/axon/libaxon_pjrt.so")}
    mod.set_axon_ntff_profile_hook = lambda h: state.__setitem__("hook", h)
    mod.get_axon_ntff_profile_hook = lambda: state["hook"]
    sys.modules["antenv.axon_hooks"] = mod
    antenv.axon_hooks = mod


tile.TileContext._drain_and_barrier = _patched_drain_and_barrier
_install_ntff_hook()

# ---------------------------------------------------------------- config

CFG_FULL = dict(NT=50000, NV=1000, E1=200000, E2=100000)


def _derive(cfg):
    d = dict(cfg)
    NT, NV = cfg["NT"], cfg["NV"]
    d["SH"] = NT // NCORE
    d["SHT"] = math.ceil(d["SH"] / P)
    d["SHP"] = d["SHT"] * P
    d["NVT"] = math.ceil(NV / P)
    d["NVP"] = d["NVT"] * P
    d["SECTION"] = d["SHP"] + NV
    d["XTILES"] = math.ceil(NT / P)
    return d


# ---------------------------------------------------------------- host prep

def _prep(inputs, cfg):
    c = _derive(cfg)
    NT, NV, E1, E2 = c["NT"], c["NV"], c["E1"], c["E2"]
    SH, SHT, SHP, NVT, NVP, SECTION = (
        c["SH"], c["SHT"], c["SHP"], c["NVT"], c["NVP"], c["SECTION"])

    inp = {k: np.asarray(v) for k, v in inputs.items()}
    f32 = np.float32

    task_feats = np.stack([
        inp["task_state_scheduled"], inp["task_state_ready"],
        inp["task_length"], inp["task_completion_time"],
        inp["task_memory_req_mb"], inp["task_cpu_req_cores"],
    ], axis=0).astype(f32)
    vm_feats = np.stack([
        inp["vm_completion_time"], inp["vm_speed"], inp["vm_energy_rate"],
        inp["vm_memory_mb"], inp["vm_available_memory_mb"],
        inp["vm_used_memory_fraction"], inp["vm_active_tasks_count"],
        inp["vm_cpu_cores"], inp["vm_available_cpu_cores"],
        inp["vm_used_cpu_fraction_cores"],
    ], axis=0).astype(f32)

    # x-moment input, pre-permuted so device DMAs are contiguous:
    # chunk c holds 16 tiles; layout [c, p, t, f8] flattened to [c, p, 128].
    xaug = np.concatenate([task_feats.T, np.ones((NT, 1), f32),
                           np.zeros((NT, 1), f32)], axis=1)
    XT = c["XTILES"]
    xaug_t = np.zeros((XT * P, 8), f32)
    xaug_t[:NT] = xaug
    NCH = math.ceil(XT / 16)
    xmom = np.zeros((NCH, P, 16, 8), f32)
    for t in range(XT):
        ch, tt = divmod(t, 16)
        xmom[ch, :, tt, :] = xaug_t[t * P:(t + 1) * P]
    xmom = xmom.reshape(NCH, P, 128)

    x_own = np.zeros((NCORE, 8, SH), f32)
    for r in range(NCORE):
        x_own[r, :6] = task_feats[:, r * SH:(r + 1) * SH]
        x_own[r, 6] = 1.0

    vm_x = np.concatenate([vm_feats, np.ones((1, NV), f32),
                           np.zeros((1, NV), f32)], axis=0)
    wscale = np.ones((8, 1), f32); wscale[4, 0] = 1e-3
    vscale = np.ones((12, 1), f32); vscale[3, 0] = 1e-3; vscale[4, 0] = 1e-3
    vm_speed_row = vm_feats[1:2, :].copy()
    vm_cpu_row = vm_feats[7:8, :].copy()

    W = {}
    W["te_W1"] = np.concatenate([inp["te_W1"], inp["te_b1"][None, :],
                                 np.zeros((1, HID), f32)], axis=0).astype(f32)
    def rowpack(w):
        w = np.asarray(w, f32)
        k = w.shape[0] // P
        return np.concatenate([w[i * P:(i + 1) * P] for i in range(k)], axis=1)

    W["te_W2"] = rowpack(inp["te_W2"])
    W["te_W3"] = rowpack(inp["te_W3"])
    W["ve_W1"] = np.concatenate([inp["ve_W1"], inp["ve_b1"][None, :],
                                 np.zeros((1, HID), f32)], axis=0).astype(f32)
    W["ve_W2"] = rowpack(inp["ve_W2"])
    W["ve_W3"] = rowpack(inp["ve_W3"])
    W["g1_Wa"] = inp["g1_Wa"].astype(f32)
    W["g1_Wb"] = rowpack(inp["g1_Wb"])
    W["g2_Wc"] = rowpack(inp["g2_Wc"])
    W["g2_Wd"] = inp["g2_Wd"].astype(f32)

    def fmaj(v, nt):
        out = np.zeros((P, nt), f32)
        v = np.asarray(v, f32)
        for i in range(nt):
            seg = v[i * P:(i + 1) * P]
            out[:len(seg), i] = seg
        return out

    W["te_g1"] = fmaj(inp["te_g1"], 2);   W["te_be1"] = fmaj(inp["te_be1"], 2)
    W["te_g2"] = fmaj(inp["te_g2"], 2);   W["te_be2"] = fmaj(inp["te_be2"], 2)
    W["te_b3f"] = fmaj(inp["te_b3"], 1)
    W["ve_g1"] = fmaj(inp["ve_g1"], 2);   W["ve_be1"] = fmaj(inp["ve_be1"], 2)
    W["ve_g2"] = fmaj(inp["ve_g2"], 2);   W["ve_be2"] = fmaj(inp["ve_be2"], 2)
    W["ve_b3f"] = fmaj(inp["ve_b3"], 1)
    W["g1_baf"] = fmaj(inp["g1_ba"], 2);  W["g1_bbf"] = fmaj(inp["g1_bb"], 2)
    W["g2_bcf"] = fmaj(inp["g2_bc"], 1);  W["g2_bdf"] = fmaj(inp["g2_bd"], 1)

    csrc = np.asarray(inp["compat_src"], np.int64)
    cdst = np.asarray(inp["compat_dst"], np.int64)
    dsrc = np.asarray(inp["dep_src"], np.int64)
    ddst = np.asarray(inp["dep_dst"], np.int64)
    c_owner = csrc // SH
    d_owner = ddst // SH

    Mmat = np.zeros((NCORE, SHP, NVP), f32)
    for r in range(NCORE):
        m = c_owner == r
        np.add.at(Mmat[r], (csrc[m] - r * SH, cdst[m]), 1.0)

    # dep blocks: grouped by local dst tile, per-tile block counts padded to
    # the max over cores (SPMD-static structure).
    dep_e = [[[] for _ in range(SHT)] for _ in range(NCORE)]
    for i in range(E2):
        r = int(d_owner[i])
        dep_e[r][(int(ddst[i]) - r * SH) // P].append(i)
    dep_nb = [max(1, math.ceil(max(len(dep_e[r][t]) for r in range(NCORE)) / P))
              for t in range(SHT)]
    B_DEP = int(np.sum(dep_nb))
    dep_gidx = np.zeros((NCORE, B_DEP * P, 1), np.int32)
    dep_drel = np.full((NCORE, P, B_DEP), -1.0, f32)
    dep_tile = []
    dep_eids = np.full((NCORE, B_DEP * P), -1, np.int64)
    b = 0
    for t in range(SHT):
        for k in range(dep_nb[t]):
            dep_tile.append(t)
            for r in range(NCORE):
                ids = dep_e[r][t][k * P:(k + 1) * P]
                for j, eid in enumerate(ids):
                    s = int(dsrc[eid])
                    dep_gidx[r, b * P + j, 0] = (s // SH) * SECTION + (s % SH)
                    dep_drel[r, j, b] = (int(ddst[eid]) - r * SH) - t * P
                    dep_eids[r, b * P + j] = eid
            b += 1
    assert b == B_DEP

    # edge-left stream: all edges grouped by local src tile of their owner.
    left_e = [[[] for _ in range(SHT)] for _ in range(NCORE)]
    for i in range(E1):
        r = int(c_owner[i])
        left_e[r][(int(csrc[i]) - r * SH) // P].append((0, i))
    for i in range(E2):
        r = int(dsrc[i] // SH)
        left_e[r][(int(dsrc[i]) - r * SH) // P].append((1, i))
    left_nb = [max(1, math.ceil(max(len(left_e[r][t]) for r in range(NCORE)) / P))
               for t in range(SHT)]
    B_LEFT = int(np.sum(left_nb))
    NQ_L = math.ceil(B_LEFT / 4)
    left_srel = np.full((NCORE, NQ_L, 512), -1.0, f32)
    left_tile = []
    left_eids = np.full((NCORE, B_LEFT * P), -1, np.int64)
    left_kind = np.zeros((NCORE, B_LEFT * P), np.int8)
    b = 0
    for t in range(SHT):
        for k in range(left_nb[t]):
            left_tile.append(t)
            for r in range(NCORE):
                ids = left_e[r][t][k * P:(k + 1) * P]
                for j, (kind, eid) in enumerate(ids):
                    s = int(csrc[eid] if kind == 0 else dsrc[eid])
                    left_srel[r, b // 4, (b % 4) * P + j] = (s % SH) - t * P
                    left_eids[r, b * P + j] = eid
                    left_kind[r, b * P + j] = kind
            b += 1
    assert b == B_LEFT

    # edge-right compat: compat edges (src owner) grouped by vm dst tile.
    rc_e = [[[] for _ in range(NVT)] for _ in range(NCORE)]
    for i in range(E1):
        rc_e[int(c_owner[i])][int(cdst[i]) // P].append(i)
    rc_nb = [max(1, math.ceil(max(len(rc_e[r][t]) for r in range(NCORE)) / P))
             for t in range(NVT)]
    B_RC = int(np.sum(rc_nb))
    NQ_RC = math.ceil(B_RC / 4)
    rc_drel = np.full((NCORE, NQ_RC, 512), -1.0, f32)
    rc_tile = []
    rc_eids = np.full((NCORE, B_RC * P), -1, np.int64)
    b = 0
    for t in range(NVT):
        for k in range(rc_nb[t]):
            rc_tile.append(t)
            for r in range(NCORE):
                ids = rc_e[r][t][k * P:(k + 1) * P]
                for j, eid in enumerate(ids):
                    rc_drel[r, b // 4, (b % 4) * P + j] = int(cdst[eid]) - t * P
                    rc_eids[r, b * P + j] = eid
            b += 1
    assert b == B_RC

    # edge-right dep: reuse dep blocks; quad layout of dep_drel for selT.
    NQ_D = math.ceil(B_DEP / 4)
    dep_drel_quad = np.full((NCORE, NQ_D, 512), -1.0, f32)
    for b in range(B_DEP):
        dep_drel_quad[:, b // 4, (b % 4) * P:(b % 4) * P + P] = dep_drel[:, :, b]

    meta = dict(cfg=c, B_DEP=B_DEP, dep_tile=dep_tile, B_LEFT=B_LEFT,
                left_tile=left_tile, B_RC=B_RC, rc_tile=rc_tile, NCH=NCH)

    per_core = []
    for r in range(NCORE):
        d = dict(
            xmom=xmom, x_own=x_own[r], vm_x=vm_x, M=Mmat[r],
            wscale=wscale, vscale=vscale, vm_speed_row=vm_speed_row,
            vm_cpu_row=vm_cpu_row,
            dep_gidx=dep_gidx[r], dep_drel=dep_drel[r],
            dep_drel_quad=dep_drel_quad[r], left_srel=left_srel[r],
            rc_drel=rc_drel[r])
        d.update({k: np.ascontiguousarray(v) for k, v in W.items()})
        per_core.append(d)

    asm = dict(meta=meta, dep_eids=dep_eids, left_eids=left_eids,
               left_kind=left_kind, rc_eids=rc_eids)
    return meta, per_core, asm


# ---------------------------------------------------------------- device

def _build(meta):
    c = meta["cfg"]
    NT, NV = c["NT"], c["NV"]
    SH, SHT, SHP, NVT, NVP, SECTION = (
        c["SH"], c["SHT"], c["SHP"], c["NVT"], c["NVP"], c["SECTION"])
    B_DEP, B_LEFT, B_RC = meta["B_DEP"], meta["B_LEFT"], meta["B_RC"]
    NCH = meta["NCH"]
    XT = c["XTILES"]
    NBL = math.ceil(SH / 512)
    # vm column chunks (matmul N <= 512)
    VCH = [(i, min(NV, i + 512)) for i in range(0, NV, 512)]

    nc = bass.Bass("TRN2", target_bir_lowering=False, debug=False)

    def ein(name, shape, dtype=F32):
        return nc.dram_tensor(name, shape, dtype, kind="ExternalInput")

    xmom_d = ein("xmom", [NCH, P, 128])
    x_own = ein("x_own", [8, SH])
    vm_x = ein("vm_x", [12, NV])
    wscale = ein("wscale", [8, 1])
    vscale = ein("vscale", [12, 1])
    vm_speed_row = ein("vm_speed_row", [1, NV])
    vm_cpu_row = ein("vm_cpu_row", [1, NV])
    M_in = ein("M", [SHP, NVP])
    dep_gidx = ein("dep_gidx", [B_DEP * P, 1], I32)
    dep_drel = ein("dep_drel", [P, B_DEP])
    dep_drel_quad = ein("dep_drel_quad", [math.ceil(B_DEP / 4), 512])
    left_srel = ein("left_srel", [math.ceil(B_LEFT / 4), 512])
    rc_drel = ein("rc_drel", [math.ceil(B_RC / 4), 512])

    te_W1 = ein("te_W1", [8, HID]);    te_W2 = ein("te_W2", [P, 2 * HID])
    te_W3 = ein("te_W3", [P, 2 * EMB])
    ve_W1 = ein("ve_W1", [12, HID]);   ve_W2 = ein("ve_W2", [P, 2 * HID])
    ve_W3 = ein("ve_W3", [P, 2 * EMB])
    te_g1 = ein("te_g1", [P, 2]);      te_be1 = ein("te_be1", [P, 2])
    te_g2 = ein("te_g2", [P, 2]);      te_be2 = ein("te_be2", [P, 2])
    te_b3f = ein("te_b3f", [P, 1])
    ve_g1 = ein("ve_g1", [P, 2]);      ve_be1 = ein("ve_be1", [P, 2])
    ve_g2 = ein("ve_g2", [P, 2]);      ve_be2 = ein("ve_be2", [P, 2])
    ve_b3f = ein("ve_b3f", [P, 1])
    g1_Wa = ein("g1_Wa", [EMB, HID]);  g1_Wb = ein("g1_Wb", [P, 2 * HID])
    g2_Wc = ein("g2_Wc", [P, 2 * EMB]);  g2_Wd = ein("g2_Wd", [EMB, EMB])
    g1_baf = ein("g1_baf", [P, 2]);    g1_bbf = ein("g1_bbf", [P, 2])
    g2_bcf = ein("g2_bcf", [P, 1]);    g2_bdf = ein("g2_bdf", [P, 1])

    def eout(name, shape, dtype=F32):
        return nc.dram_tensor(name, shape, dtype, kind="ExternalOutput")

    o_ne = eout("o_ne", [SH, EMB])
    o_nevm = eout("o_nevm", [NV, EMB])
    o_graph = eout("o_graph", [P, 2])
    o_left = eout("o_left", [B_LEFT * P, EMB])
    o_rc = eout("o_rc", [B_RC * P, EMB])
    o_rd = eout("o_rd", [B_DEP * P, EMB])
    o_dbg = eout("o_dbg", [P, 4 * NV])

    cc1_in = nc.dram_tensor("cc1_in", [SECTION, EMB], F32)
    cc1_out = nc.dram_tensor("cc1_out", [NCORE * SECTION, EMB], F32)
    cc2_in = nc.dram_tensor("cc2_in", [SECTION, EMB], F32)
    cc2_out = nc.dram_tensor("cc2_out", [NCORE * SECTION, EMB], F32)
    st_in = nc.dram_tensor("st_in", [P, 4], F32)
    st_out = nc.dram_tensor("st_out", [P, 4], F32)
    netask_rm = nc.dram_tensor("netask_rm", [SHP, EMB], F32)
    p2_d = nc.dram_tensor("p2_d", [P, 2 * SH], F32)
    th_fm = nc.dram_tensor("th_fm", [P, SH], F32)
    y2_fm = nc.dram_tensor("y2_fm", [P, SH], F32)
    nevm_rm = nc.dram_tensor("nevm_rm", [NVP, EMB], F32)

    RG = [list(range(NCORE))]
    AX = mybir.AxisListType.X
    ALU = mybir.AluOpType
    ACTF = mybir.ActivationFunctionType

    with tile.TileContext(nc) as tc, contextlib.ExitStack() as ctx:
        const = ctx.enter_context(tc.tile_pool(name="const", bufs=1))
        sb = ctx.enter_context(tc.tile_pool(name="sb", bufs=2))
        sb3 = ctx.enter_context(tc.tile_pool(name="sb3", bufs=3))
        ps = ctx.enter_context(tc.tile_pool(name="ps", bufs=2, space="PSUM"))
        ps_acc = ctx.enter_context(
            tc.tile_pool(name="ps_acc", bufs=2, space="PSUM"))
        ps_vg = ctx.enter_context(
            tc.tile_pool(name="ps_vg", bufs=1, space="PSUM"))

        _scope_ids = {}

        def scope_in(name):
            _scope_ids[name] = nc.enter_named_scope(name, False)[0]

        def scope_out(name):
            nc.leave_named_scope(name, _scope_ids.pop(name), False)

        # -------- constants
        ident = const.tile([P, P], F32, tag="ident")
        make_identity(nc, ident[:])
        iota_row_i = const.tile([P, P], I32, tag="ioti")
        nc.gpsimd.iota(iota_row_i[:], pattern=[[1, P]], channel_multiplier=0)
        iota_row = const.tile([P, P], F32, tag="iotr")
        nc.vector.tensor_copy(out=iota_row[:], in_=iota_row_i[:])
        iota_col_i = const.tile([P, P], I32, tag="iotci")
        nc.gpsimd.iota(iota_col_i[:], pattern=[[0, P]], channel_multiplier=1)
        iota_col = const.tile([P, P], F32, tag="iotc")
        nc.vector.tensor_copy(out=iota_col[:], in_=iota_col_i[:])
        ones_row = const.tile([1, 512], F32, tag="ones")
        nc.gpsimd.memset(ones_row[:], 1.0)
        ones8 = const.tile([8, 1], F32, tag="ones8")
        nc.gpsimd.memset(ones8[:], 1.0)

        def load(t, tag):
            tl = const.tile(list(t.shape), t.dtype, tag=tag)
            nc.sync.dma_start(out=tl[:], in_=t[:])
            return tl

        w_te1 = load(te_W1, "wte1"); w_te2 = load(te_W2, "wte2")
        w_te3 = load(te_W3, "wte3")
        w_ve1 = load(ve_W1, "wve1"); w_ve2 = load(ve_W2, "wve2")
        w_ve3 = load(ve_W3, "wve3")
        w_a = load(g1_Wa, "wa"); w_b = load(g1_Wb, "wb")
        w_c = load(g2_Wc, "wc"); w_d = load(g2_Wd, "wd")
        v_te_g1 = load(te_g1, "vg1"); v_te_be1 = load(te_be1, "vb1")
        v_te_g2 = load(te_g2, "vg2"); v_te_be2 = load(te_be2, "vb2")
        v_te_b3 = load(te_b3f, "vb3")
        v_ve_g1 = load(ve_g1, "wg1"); v_ve_be1 = load(ve_be1, "wb1")
        v_ve_g2 = load(ve_g2, "wg2"); v_ve_be2 = load(ve_be2, "wb2")
        v_ve_b3 = load(ve_b3f, "wb3v")
        v_ba = load(g1_baf, "vba"); v_bb = load(g1_bbf, "vbb")
        v_bc = load(g2_bcf, "vbc"); v_bd = load(g2_bdf, "vbd")

        scope_in("pre")
        # -------- vm input transform + maxc (offset-0 partition ops only;
        # partition placement done via DMA)
        vmx_raw = const.tile([12, NV], F32, tag="vmxr")
        nc.sync.dma_start(out=vmx_raw[:], in_=vm_x[:])
        vcpu = const.tile([1, NV], F32, tag="vcpu")
        nc.sync.dma_start(out=vcpu[:], in_=vm_cpu_row[:])
        maxc = const.tile([1, 1], F32, tag="maxc")
        nc.vector.reduce_max(out=maxc[:], in_=vcpu[:], axis=AX)
        maxc1 = const.tile([1, 1], F32, tag="maxc1")
        nc.vector.tensor_scalar_max(maxc1[:], maxc[:], 1.0)
        rmaxc = const.tile([1, 1], F32, tag="rmaxc")
        nc.vector.reciprocal(out=rmaxc[:], in_=maxc1[:])
        rm8 = const.tile([8, 1], F32, tag="rm8")
        nc.gpsimd.memset(rm8[:], 1.0)
        nc.sync.dma_start(out=rm8[5:6, :], in_=rmaxc[0:1, 0:1])
        wsc = const.tile([8, 1], F32, tag="wsc")
        nc.sync.dma_start(out=wsc[:], in_=wscale[:])
        nc.vector.tensor_tensor(out=wsc[:], in0=wsc[:], in1=rm8[:], op=ALU.mult)
        w_te1s = const.tile([8, HID], F32, tag="wte1s")
        nc.vector.tensor_scalar_mul(w_te1s[:], w_te1[:], wsc[:, 0:1])
        rm12 = const.tile([12, 1], F32, tag="rm12")
        nc.gpsimd.memset(rm12[:], 1.0)
        nc.sync.dma_start(out=rm12[7:8, :], in_=rmaxc[0:1, 0:1])
        nc.sync.dma_start(out=rm12[8:9, :], in_=rmaxc[0:1, 0:1])
        vsc = const.tile([12, 1], F32, tag="vsc")
        nc.sync.dma_start(out=vsc[:], in_=vscale[:])
        nc.vector.tensor_tensor(out=vsc[:], in0=vsc[:], in1=rm12[:], op=ALU.mult)
        vmx = const.tile([12, NV], F32, tag="vmx")
        nc.vector.tensor_scalar_mul(vmx[:], vmx_raw[:], vsc[:, 0:1])
        spd = const.tile([1, NV], F32, tag="spd")
        nc.sync.dma_start(out=spd[:], in_=vm_speed_row[:])
        nc.vector.tensor_scalar_add(spd[:], spd[:], 1e-8)
        nc.vector.reciprocal(out=spd[:], in_=spd[:])
        nc.sync.dma_start(out=vmx[1:2, :], in_=spd[0:1, :])

        # -------- x moments: Caug = sum over rows of [x | 1] outer products
        cmom_ps = ps_acc.tile([8, 8], F32, space="PSUM", tag="acc")
        for ch in range(NCH):
            xm = sb3.tile([P, 128], F32, tag="xm")
            nc.sync.dma_start(out=xm[:], in_=xmom_d[ch])
            for t in range(16):
                gt = ch * 16 + t
                if gt >= XT:
                    break
                nc.tensor.matmul(out=cmom_ps[:], lhsT=xm[:, t * 8:t * 8 + 8],
                                 rhs=xm[:, t * 8:t * 8 + 8],
                                 start=(gt == 0), stop=(gt == XT - 1))
        caug = const.tile([8, 8], F32, tag="caug")
        nc.vector.tensor_copy(out=caug[:], in_=cmom_ps[:])

        # BN1 scale/bias from moments (scaled W1')
        cw_ps = ps.tile([8, HID], F32, space="PSUM", tag="w512")
        nc.tensor.matmul(out=cw_ps[:], lhsT=caug[:], rhs=w_te1s[:],
                         start=True, stop=True)
        ep = sb.tile([8, HID], F32, tag="ep")
        nc.vector.tensor_tensor(out=ep[:], in0=w_te1s[:], in1=cw_ps[:],
                                op=ALU.mult)
        bn1_s = const.tile([P, 2], F32, tag="bn1s")
        bn1_b = const.tile([P, 2], F32, tag="bn1b")
        ep2 = sb.tile([P, 2], F32, tag="ep2")
        mean1 = sb.tile([P, 2], F32, tag="mean1")
        for f in range(2):
            pp = ps.tile([P, 1], F32, space="PSUM", tag="w128")
            nc.tensor.matmul(out=pp[:], lhsT=ep[:, f * P:(f + 1) * P],
                             rhs=ones8[:], start=True, stop=True)
            nc.scalar.activation(out=ep2[:, f:f + 1], in_=pp[:],
                                 func=ACTF.Copy, scale=1.0 / NT)
            pp2 = ps.tile([P, 1], F32, space="PSUM", tag="w128")
            nc.tensor.matmul(out=pp2[:], lhsT=w_te1s[:, f * P:(f + 1) * P],
                             rhs=caug[:, 6:7], start=True, stop=True)
            nc.scalar.activation(out=mean1[:, f:f + 1], in_=pp2[:],
                                 func=ACTF.Copy, scale=1.0 / NT)
        var1 = sb.tile([P, 2], F32, tag="var1")
        nc.vector.tensor_tensor(out=var1[:], in0=mean1[:], in1=mean1[:],
                                op=ALU.mult)
        nc.vector.tensor_tensor(out=var1[:], in0=ep2[:], in1=var1[:],
                                op=ALU.subtract)
        nc.vector.tensor_scalar_add(var1[:], var1[:], 1e-5)
        nc.scalar.activation(out=var1[:], in_=var1[:], func=ACTF.Sqrt)
        nc.vector.reciprocal(out=var1[:], in_=var1[:])
        nc.vector.tensor_tensor(out=bn1_s[:], in0=v_te_g1[:], in1=var1[:],
                                op=ALU.mult)
        nc.vector.tensor_tensor(out=bn1_b[:], in0=mean1[:], in1=bn1_s[:],
                                op=ALU.mult)
        nc.vector.tensor_tensor(out=bn1_b[:], in0=v_te_be1[:], in1=bn1_b[:],
                                op=ALU.subtract)

        scope_out("pre")
        scope_in("enc")
        # -------- task encoder to p2 (raw), with BN2 stat accumulation
        stats = const.tile([P, 4], F32, tag="stats")
        nc.gpsimd.memset(stats[:], 0.0)
        for nb in range(NBL):
            n0 = nb * 512
            n1 = min(SH, n0 + 512)
            w = n1 - n0
            xob = sb.tile([8, 512], F32, tag="xob")
            nc.sync.dma_start(out=xob[:, :w], in_=x_own[:, n0:n1])
            h1b = sb.tile([P, 2 * 512], F32, tag="h1b")
            for f in range(2):
                pp = ps.tile([P, 512], F32, space="PSUM", tag="w512")
                nc.tensor.matmul(out=pp[:, :w],
                                 lhsT=w_te1s[:, f * P:(f + 1) * P],
                                 rhs=xob[:, :w], start=True, stop=True)
                nc.scalar.activation(out=h1b[:, f * 512:f * 512 + w],
                                     in_=pp[:, :w], func=ACTF.Relu,
                                     bias=bn1_b[:, f:f + 1],
                                     scale=bn1_s[:, f:f + 1])
            for f in range(2):
                pp = ps.tile([P, 512], F32, space="PSUM", tag="w512")
                for k in range(2):
                    nc.tensor.matmul(
                        out=pp[:, :w],
                        lhsT=w_te2[:, k * HID + f * P:k * HID + (f + 1) * P],
                        rhs=h1b[:, k * 512:k * 512 + w],
                        start=(k == 0), stop=(k == 1))
                p2b = sb.tile([P, 512], F32, tag="p2b")
                nc.vector.tensor_copy(out=p2b[:, :w], in_=pp[:, :w])
                nc.sync.dma_start(out=p2_d[:, f * SH + n0:f * SH + n1],
                                  in_=p2b[:, :w])
                r1 = sb.tile([P, 1], F32, tag="str1")
                nc.vector.reduce_sum(out=r1[:], in_=p2b[:, :w], axis=AX)
                nc.vector.tensor_add(out=stats[:, f:f + 1],
                                     in0=stats[:, f:f + 1], in1=r1[:])
                sqb = sb.tile([P, 512], F32, tag="sqb")
                nc.vector.tensor_tensor(out=sqb[:, :w], in0=p2b[:, :w],
                                        in1=p2b[:, :w], op=ALU.mult)
                r2 = sb.tile([P, 1], F32, tag="str1")
                nc.vector.reduce_sum(out=r2[:], in_=sqb[:, :w], axis=AX)
                nc.vector.tensor_add(out=stats[:, 2 + f:3 + f],
                                     in0=stats[:, 2 + f:3 + f], in1=r2[:])

        scope_out("enc")
        scope_in("s1_ar")
        # -------- S1: stats AllReduce
        nc.sync.dma_start(out=st_in[:], in_=stats[:])
        nc.gpsimd.collective_compute("AllReduce", ALU.add, replica_groups=RG,
                                     ins=[st_in[:]], outs=[st_out[:]])
        st_sb = sb.tile([P, 4], F32, tag="stsb")
        nc.sync.dma_start(out=st_sb[:], in_=st_out[:])
        bn2_s = const.tile([P, 2], F32, tag="bn2s")
        bn2_b = const.tile([P, 2], F32, tag="bn2b")
        mean2 = sb.tile([P, 2], F32, tag="mean2")
        var2 = sb.tile([P, 2], F32, tag="var2")
        nc.scalar.activation(out=mean2[:], in_=st_sb[:, 0:2], func=ACTF.Copy,
                             scale=1.0 / NT)
        nc.scalar.activation(out=var2[:], in_=st_sb[:, 2:4], func=ACTF.Copy,
                             scale=1.0 / NT)
        m2sq = sb.tile([P, 2], F32, tag="m2sq")
        nc.vector.tensor_tensor(out=m2sq[:], in0=mean2[:], in1=mean2[:],
                                op=ALU.mult)
        nc.vector.tensor_tensor(out=var2[:], in0=var2[:], in1=m2sq[:],
                                op=ALU.subtract)
        nc.vector.tensor_scalar_add(var2[:], var2[:], 1e-5)
        nc.scalar.activation(out=var2[:], in_=var2[:], func=ACTF.Sqrt)
        nc.vector.reciprocal(out=var2[:], in_=var2[:])
        nc.vector.tensor_tensor(out=bn2_s[:], in0=v_te_g2[:], in1=var2[:],
                                op=ALU.mult)
        nc.vector.tensor_tensor(out=bn2_b[:], in0=mean2[:], in1=bn2_s[:],
                                op=ALU.mult)
        nc.vector.tensor_tensor(out=bn2_b[:], in0=v_te_be2[:], in1=bn2_b[:],
                                op=ALU.subtract)

        # transpose helper: f-major SBUF block [128, <=512] -> row-major
        # DRAM rows (one or two destinations)
        def transpose_block(drams, blk, n0, w):
            for k in range(math.ceil(w / P)):
                c0 = k * P
                c1 = min(w, c0 + P)
                w2 = c1 - c0
                tp = ps.tile([P, P], F32, space="PSUM", tag="w128")
                nc.tensor.transpose(out=tp[:w2, :], in_=blk[:, c0:c1],
                                    identity=ident[:])
                so = sb3.tile([P, P], F32, tag="tpo")
                nc.scalar.activation(out=so[:w2, :], in_=tp[:w2, :],
                                     func=ACTF.Copy)
                for dram, row0 in drams:
                    nc.sync.dma_start(
                        out=dram[row0 + n0 + c0:row0 + n0 + c1, :],
                        in_=so[:w2, :])

        scope_out("s1_ar")
        scope_in("enc2")
        # -------- h2 = relu(bn2(p2)); task_h = W3^T h2 + b3 (f-major)
        for nb in range(NBL):
            n0 = nb * 512
            n1 = min(SH, n0 + 512)
            w = n1 - n0
            h2b = sb.tile([P, 2 * 512], F32, tag="h1b")
            for f in range(2):
                p2l = sb.tile([P, 512], F32, tag="p2b")
                nc.sync.dma_start(out=p2l[:, :w],
                                  in_=p2_d[:, f * SH + n0:f * SH + n1])
                nc.scalar.activation(out=h2b[:, f * 512:f * 512 + w],
                                     in_=p2l[:, :w],
                                     func=ACTF.Relu, bias=bn2_b[:, f:f + 1],
                                     scale=bn2_s[:, f:f + 1])
            pp = ps.tile([P, 512], F32, space="PSUM", tag="w512")
            for k in range(2):
                nc.tensor.matmul(out=pp[:, :w], lhsT=w_te3[:, k * EMB:(k + 1) * EMB],
                                 rhs=h2b[:, k * 512:k * 512 + w],
                                 start=(k == 0), stop=(k == 1))
            thb = sb.tile([P, 512], F32, tag="thb")
            nc.vector.tensor_tensor(
                out=thb[:, :w], in0=pp[:, :w],
                in1=v_te_b3[:, 0:1].to_broadcast([P, w]), op=ALU.add)
            nc.sync.dma_start(out=th_fm[:, n0:n1], in_=thb[:, :w])
            transpose_block([(cc1_in, 0)], thb, n0, w)
        if SHP > SH:
            zpad = sb.tile([SHP - SH, EMB], F32, tag="zpad")
            nc.gpsimd.memset(zpad[:], 0.0)
            nc.sync.dma_start(out=cc1_in[SH:SHP, :], in_=zpad[:])

        scope_out("enc2")
        scope_in("vmenc")
        # -------- vm encoder (replicated, local BN)
        def bn_local(praw, dst, f, n, g, be, relu):
            s1 = sb.tile([P, 1], F32, tag="vmst_s1")
            nc.vector.reduce_sum(out=s1[:], in_=praw[:, :n], axis=AX)
            sqv = sb.tile([P, NV], F32, tag="vmsq")
            nc.vector.tensor_tensor(out=sqv[:, :n], in0=praw[:, :n],
                                    in1=praw[:, :n], op=ALU.mult)
            s2 = sb.tile([P, 1], F32, tag="vmst_s2")
            nc.vector.reduce_sum(out=s2[:], in_=sqv[:, :n], axis=AX)
            mu = sb.tile([P, 1], F32, tag="vmst_mu")
            nc.scalar.activation(out=mu[:], in_=s1[:], func=ACTF.Copy,
                                 scale=1.0 / n)
            e2 = sb.tile([P, 1], F32, tag="vmst_e2")
            nc.scalar.activation(out=e2[:], in_=s2[:], func=ACTF.Copy,
                                 scale=1.0 / n)
            vv = sb.tile([P, 1], F32, tag="vmst_vv")
            nc.vector.tensor_tensor(out=vv[:], in0=mu[:], in1=mu[:], op=ALU.mult)
            nc.vector.tensor_tensor(out=vv[:], in0=e2[:], in1=vv[:],
                                    op=ALU.subtract)
            nc.vector.tensor_scalar_add(vv[:], vv[:], 1e-5)
            nc.scalar.activation(out=vv[:], in_=vv[:], func=ACTF.Sqrt)
            nc.vector.reciprocal(out=vv[:], in_=vv[:])
            sc = sb.tile([P, 1], F32, tag="vmst_sc")
            nc.vector.tensor_tensor(out=sc[:], in0=g[:, f:f + 1], in1=vv[:],
                                    op=ALU.mult)
            bi = sb.tile([P, 1], F32, tag="vmst_bi")
            nc.vector.tensor_tensor(out=bi[:], in0=mu[:], in1=sc[:], op=ALU.mult)
            nc.vector.tensor_tensor(out=bi[:], in0=be[:, f:f + 1], in1=bi[:],
                                    op=ALU.subtract)
            nc.scalar.activation(out=dst, in_=praw[:, :n],
                                 func=ACTF.Relu if relu else ACTF.Copy,
                                 bias=bi[:, 0:1], scale=sc[:, 0:1])

        def mm_chunks(out_sb, lhsT_list, rhs_sb, rhs_off, n):
            """psum-chunked matmul: out_sb[:, :n] = sum_k lhsT_k^T rhs_k."""
            for (c0, c1) in [(i, min(n, i + 512)) for i in range(0, n, 512)]:
                pp = ps.tile([P, 512], F32, space="PSUM", tag="w512")
                for ki, (lh, rh) in enumerate(zip(lhsT_list, rhs_off)):
                    nc.tensor.matmul(out=pp[:, :c1 - c0], lhsT=lh,
                                     rhs=rhs_sb[:, rh + c0:rh + c1],
                                     start=(ki == 0),
                                     stop=(ki == len(lhsT_list) - 1))
                yield (c0, c1, pp)

        vp1raw = const.tile([P, 2 * NV], F32, tag="vmX0")
        for f in range(2):
            for c0, c1, pp in mm_chunks(None, [w_ve1[:, f * P:(f + 1) * P]],
                                        vmx, [0], NV):
                nc.vector.tensor_copy(out=vp1raw[:, f * NV + c0:f * NV + c1],
                                      in_=pp[:, :c1 - c0])
        vh1 = const.tile([P, 2 * NV], F32, tag="vmB")
        for f in range(2):
            bn_local(vp1raw[:, f * NV:(f + 1) * NV], vh1[:, f * NV:(f + 1) * NV],
                     f, NV, v_ve_g1, v_ve_be1, True)
        vp2raw = const.tile([P, 2 * NV], F32, tag="vmX0")
        for f in range(2):
            for c0, c1, pp in mm_chunks(
                    None,
                    [w_ve2[:, k * HID + f * P:k * HID + (f + 1) * P] for k in range(2)],
                    vh1, [0, NV], NV):
                nc.vector.tensor_copy(out=vp2raw[:, f * NV + c0:f * NV + c1],
                                      in_=pp[:, :c1 - c0])
        vh2 = const.tile([P, 2 * NV], F32, tag="vmC")
        for f in range(2):
            bn_local(vp2raw[:, f * NV:(f + 1) * NV], vh2[:, f * NV:(f + 1) * NV],
                     f, NV, v_ve_g2, v_ve_be2, True)
        vmh_t = const.tile([P, NV], F32, tag="vmh")
        for c0, c1, pp in mm_chunks(
                None, [w_ve3[:, k * EMB:(k + 1) * EMB] for k in range(2)],
                vh2, [0, NV], NV):
            nc.vector.tensor_tensor(
                out=vmh_t[:, c0:c1], in0=pp[:, :c1 - c0],
                in1=v_ve_b3[:, 0:1].to_broadcast([P, c1 - c0]), op=ALU.add)

        scope_out("vmenc")
        scope_in("compat1")
        # -------- compat partial agg via count matrix M
        def compat_agg(table_dram, out_cc):
            # PSUM accumulation groups must not interleave on a bank:
            # accumulate each vm tile contiguously (t inner), table cached.
            tcache = const.tile([P, SHT * EMB], F32, tag="tcache")
            for t in range(SHT):
                nc.sync.dma_start(out=tcache[:, t * EMB:(t + 1) * EMB],
                                  in_=table_dram[t * P:(t + 1) * P, :])
            for vt in range(NVT):
                pv_t = ps_vg.tile([P, 512], F32, space="PSUM",
                                  tag=f"vg{vt % 2}")
                for t in range(SHT):
                    mtile = sb3.tile([P, P], F32, tag="cmtile")
                    nc.sync.dma_start(
                        out=mtile[:],
                        in_=M_in[t * P:(t + 1) * P, vt * P:(vt + 1) * P])
                    nc.tensor.matmul(out=pv_t[:, :EMB], lhsT=mtile[:],
                                     rhs=tcache[:, t * EMB:(t + 1) * EMB],
                                     start=(t == 0), stop=(t == SHT - 1))
                so = sb3.tile([P, EMB], F32, tag="vaggout")
                nc.scalar.activation(out=so[:], in_=pv_t[:, :EMB],
                                     func=ACTF.Copy)
                r0 = SHP + vt * P
                r1 = min(SHP + NV, r0 + P)
                if r1 > r0:
                    nc.sync.dma_start(out=out_cc[r0:r1, :], in_=so[:r1 - r0, :])

        compat_agg(cc1_in, cc1_in)

        scope_out("compat1")
        scope_in("s2_ag")
        # -------- S2: AllGather (task_h || vm partials)
        nc.gpsimd.collective_compute("AllGather", ALU.bypass, replica_groups=RG,
                                     ins=[cc1_in[:]], outs=[cc1_out[:]])

        def vm_agg_reduce(cc_out_t, tag):
            acc = const.tile([P, NVT * EMB], F32, tag=tag)
            for r in range(NCORE):
                part = sb.tile([P, NVT * EMB], F32, tag="vmaggld")
                for vt in range(NVT):
                    r0 = r * SECTION + SHP + vt * P
                    r1 = min(r * SECTION + SHP + NV, r0 + P)
                    if r1 <= r0:
                        continue
                    nc.sync.dma_start(
                        out=part[:r1 - r0, vt * EMB:(vt + 1) * EMB],
                        in_=cc_out_t[r0:r1, :])
                if r == 0:
                    nc.vector.tensor_copy(out=acc[:], in_=part[:])
                else:
                    nc.vector.tensor_add(out=acc[:], in0=acc[:], in1=part[:])
            agg_t = const.tile([P, NVT * P], F32, tag=tag + "t")
            for vt in range(NVT):
                tp = ps.tile([P, P], F32, space="PSUM", tag="w128")
                nc.tensor.transpose(out=tp[:], in_=acc[:, vt * EMB:(vt + 1) * EMB],
                                    identity=ident[:])
                nc.scalar.activation(out=agg_t[:, vt * P:(vt + 1) * P],
                                     in_=tp[:], func=ACTF.Copy)
            return agg_t

        vm_agg1_t = vm_agg_reduce(cc1_out, "vagg")

        scope_out("s2_ag")
        scope_in("layer1")
        # -------- dep aggregation + fused MLP consumption
        dep_tile_list = meta["dep_tile"]
        blocks_of_tile = [[] for _ in range(SHT)]
        for b, t in enumerate(dep_tile_list):
            blocks_of_tile[t].append(b)
        drel_sb = const.tile([P, B_DEP], F32, tag="drel")
        nc.sync.dma_start(out=drel_sb[:], in_=dep_drel[:])

        def dep_agg_consume(cc_out_t, x_fm, consume):
            for nb2 in range(NBL):
                n0 = nb2 * 512
                n1 = min(SH, n0 + 512)
                w = n1 - n0
                pa = ps_acc.tile([P, 512], F32, space="PSUM", tag="acc")
                t0 = nb2 * 4
                for tt in range(t0, min(SHT, t0 + 4)):
                    col0 = (tt - t0) * P
                    blks = blocks_of_tile[tt]
                    for bi, b in enumerate(blks):
                        it = sb3.tile([P, 1], I32, tag="didx")
                        nc.sync.dma_start(out=it[:],
                                          in_=dep_gidx[b * P:(b + 1) * P, :])
                        gt = sb3.tile([P, EMB], F32, tag="dgather")
                        nc.gpsimd.indirect_dma_start(
                            out=gt[:], out_offset=None, in_=cc_out_t[:, :],
                            in_offset=bass.IndirectOffsetOnAxis(
                                ap=it[:, :1], axis=0))
                        sel = sb3.tile([P, P], F32, tag="dsel")
                        nc.vector.tensor_scalar(
                            out=sel[:], in0=iota_row[:],
                            scalar1=drel_sb[:, b:b + 1], scalar2=None,
                            op0=ALU.is_equal)
                        nc.tensor.matmul(out=pa[:, col0:col0 + P], lhsT=gt[:],
                                         rhs=sel[:], start=(bi == 0),
                                         stop=(bi == len(blks) - 1))
                xl = sb.tile([P, 512], F32, tag="xl")
                nc.sync.dma_start(out=xl[:, :w], in_=x_fm[:, n0:n1])
                zt = sb.tile([P, 512], F32, tag="zt")
                nc.vector.tensor_tensor(out=zt[:, :w], in0=xl[:, :w],
                                        in1=pa[:, :w], op=ALU.add)
                consume(nb2, zt, w)

        # L1 consume: t1 = relu(Wa^T z + ba); h = relu(Wb^T t1 + bb);
        # y2 = Wc^T h  (all per 512-block, nothing persisted)

        def l1_consume(nb2, zt, w):
            n0 = nb2 * 512
            t1 = sb.tile([P, 2 * 512], F32, tag="t1")
            for f in range(2):
                pp = ps.tile([P, 512], F32, space="PSUM", tag="w512")
                nc.tensor.matmul(out=pp[:, :w], lhsT=w_a[:, f * P:(f + 1) * P],
                                 rhs=zt[:, :w], start=True, stop=True)
                nc.scalar.activation(out=t1[:, f * 512:f * 512 + w],
                                     in_=pp[:, :w], func=ACTF.Relu,
                                     bias=v_ba[:, f:f + 1])
            hb = sb.tile([P, 2 * 512], F32, tag="hb")
            for f in range(2):
                pp = ps.tile([P, 512], F32, space="PSUM", tag="w512")
                for k in range(2):
                    nc.tensor.matmul(
                        out=pp[:, :w],
                        lhsT=w_b[:, k * HID + f * P:k * HID + (f + 1) * P],
                        rhs=t1[:, k * 512:k * 512 + w],
                        start=(k == 0), stop=(k == 1))
                nc.scalar.activation(out=hb[:, f * 512:f * 512 + w],
                                     in_=pp[:, :w], func=ACTF.Relu,
                                     bias=v_bb[:, f:f + 1])
            pp = ps.tile([P, 512], F32, space="PSUM", tag="w512")
            for k in range(2):
                nc.tensor.matmul(out=pp[:, :w], lhsT=w_c[:, k * EMB:(k + 1) * EMB],
                                 rhs=hb[:, k * 512:k * 512 + w],
                                 start=(k == 0), stop=(k == 1))
            y2b = sb.tile([P, 512], F32, tag="thb")
            nc.vector.tensor_copy(out=y2b[:, :w], in_=pp[:, :w])
            nc.sync.dma_start(out=y2_fm[:, n0:n0 + w], in_=y2b[:, :w])
            transpose_block([(cc2_in, 0)], y2b, n0, w)

        dep_agg_consume(cc1_out, th_fm, l1_consume)

        # vm L1 + y2vm
        hvm_t = const.tile([P, 2 * NV], F32, tag="vmB")
        zvm = sb.tile([P, NV], F32, tag="zvm")
        nc.vector.tensor_add(out=zvm[:], in0=vmh_t[:], in1=vm_agg1_t[:, :NV])
        t1v = const.tile([P, 2 * NV], F32, tag="vmC")
        for f in range(2):
            for c0, c1, pp in mm_chunks(None, [w_a[:, f * P:(f + 1) * P]],
                                        zvm, [0], NV):
                nc.scalar.activation(out=t1v[:, f * NV + c0:f * NV + c1],
                                     in_=pp[:, :c1 - c0], func=ACTF.Relu,
                                     bias=v_ba[:, f:f + 1])
        for f in range(2):
            for c0, c1, pp in mm_chunks(
                    None,
                    [w_b[:, k * HID + f * P:k * HID + (f + 1) * P] for k in range(2)],
                    t1v, [0, NV], NV):
                nc.scalar.activation(out=hvm_t[:, f * NV + c0:f * NV + c1],
                                     in_=pp[:, :c1 - c0], func=ACTF.Relu,
                                     bias=v_bb[:, f:f + 1])
        y2vm_t = const.tile([P, NV], F32, tag="y2vm")
        for c0, c1, pp in mm_chunks(
                None, [w_c[:, k * EMB:(k + 1) * EMB] for k in range(2)],
                hvm_t, [0, NV], NV):
            nc.vector.tensor_copy(out=y2vm_t[:, c0:c1], in_=pp[:, :c1 - c0])

        scope_out("layer1")
        scope_in("cc2")
        # cc2: y2 row-major + L2 vm partials, AllGather
        if SHP > SH:
            zpad2 = sb.tile([SHP - SH, EMB], F32, tag="zpad")
            nc.gpsimd.memset(zpad2[:], 0.0)
            nc.sync.dma_start(out=cc2_in[SH:SHP, :], in_=zpad2[:])
        compat_agg(cc2_in, cc2_in)
        nc.gpsimd.collective_compute("AllGather", ALU.bypass, replica_groups=RG,
                                     ins=[cc2_in[:]], outs=[cc2_out[:]])
        scope_out("cc2")
        scope_in("layer2")
        vm_agg2_t = vm_agg_reduce(cc2_out, "vagg")

        nc.sync.dma_start(out=o_dbg[:, 0:NV], in_=vmh_t[:])
        nc.sync.dma_start(out=o_dbg[:, NV:2 * NV], in_=vm_agg1_t[:, :NV])
        nc.sync.dma_start(out=o_dbg[:, 2 * NV:3 * NV], in_=y2vm_t[:])
        nc.sync.dma_start(out=o_dbg[:, 3 * NV:4 * NV], in_=vm_agg2_t[:, :NV])

        # L2 consume: t2 = relu(z + bc); ne = Wd^T t2 + bd; outputs per block
        gacc = const.tile([P, 1], F32, tag="gacc")
        nc.gpsimd.memset(gacc[:], 0.0)

        def l2_consume(nb2, zt, w):
            n0 = nb2 * 512
            t2 = sb.tile([P, 512], F32, tag="t2")
            nc.scalar.activation(out=t2[:, :w], in_=zt[:, :w], func=ACTF.Relu,
                                 bias=v_bc[:, 0:1])
            pp = ps.tile([P, 512], F32, space="PSUM", tag="w512")
            nc.tensor.matmul(out=pp[:, :w], lhsT=w_d[:], rhs=t2[:, :w],
                             start=True, stop=True)
            neb = sb.tile([P, 512], F32, tag="thb")
            nc.vector.tensor_tensor(
                out=neb[:, :w], in0=pp[:, :w],
                in1=v_bd[:, 0:1].to_broadcast([P, w]), op=ALU.add)
            gb = sb.tile([P, 1], F32, tag="str1")
            nc.vector.reduce_sum(out=gb[:], in_=neb[:, :w], axis=AX)
            nc.vector.tensor_add(out=gacc[:], in0=gacc[:], in1=gb[:])
            transpose_block([(o_ne, 0), (netask_rm, 0)], neb, n0, w)

        dep_agg_consume(cc2_out, y2_fm, l2_consume)

        # vm L2
        nevm_t = const.tile([P, NV], F32, tag="nevm")
        zv2 = sb.tile([P, NV], F32, tag="zv2")
        nc.vector.tensor_add(out=zv2[:], in0=y2vm_t[:], in1=vm_agg2_t[:, :NV])
        nc.scalar.activation(out=zv2[:], in_=zv2[:], func=ACTF.Relu,
                             bias=v_bc[:, 0:1])
        for c0, c1, pp in mm_chunks(None, [w_d[:]], zv2, [0], NV):
            nc.vector.tensor_tensor(
                out=nevm_t[:, c0:c1], in0=pp[:, :c1 - c0],
                in1=v_bd[:, 0:1].to_broadcast([P, c1 - c0]), op=ALU.add)

        # node embedding outputs (vm part) + graph partials
        if SHP > SH:
            zp3 = sb.tile([SHP - SH, EMB], F32, tag="zpad")
            nc.gpsimd.memset(zp3[:], 0.0)
            nc.sync.dma_start(out=netask_rm[SH:SHP, :], in_=zp3[:])
        for vb in range(math.ceil(NV / 512)):
            n0 = vb * 512
            n1 = min(NV, n0 + 512)
            transpose_block([(o_nevm, 0), (nevm_rm, 0)],
                            nevm_t[:, n0:n1], n0, n1 - n0)
        if NVP > NV:
            zp4 = sb.tile([NVP - NV, EMB], F32, tag="zpad")
            nc.gpsimd.memset(zp4[:], 0.0)
            nc.sync.dma_start(out=nevm_rm[NV:NVP, :], in_=zp4[:])

        gp = sb.tile([P, 2], F32, tag="gp")
        nc.vector.tensor_copy(out=gp[:, 0:1], in_=gacc[:])
        nc.vector.reduce_sum(out=gp[:, 1:2], in_=nevm_t[:, :NV], axis=AX)
        nc.sync.dma_start(out=o_graph[:], in_=gp[:])

        scope_out("layer2")
        scope_in("edges")
        # -------- edge expansions
        def expand_stream(rel_quad_dram, n_blocks, tile_list, table_dram,
                          out_dram):
            NQ = math.ceil(n_blocks / 4)
            for q in range(NQ):
                qr = sb.tile([1, 512], F32, tag="qr")
                nc.sync.dma_start(out=qr[:], in_=rel_quad_dram[q:q + 1, :])
                bc = ps.tile([P, 512], F32, space="PSUM", tag="w512")
                nc.tensor.matmul(out=bc[:], lhsT=ones_row[:, 0:P], rhs=qr[:],
                                 start=True, stop=True)
                for j in range(4):
                    b = q * 4 + j
                    if b >= n_blocks:
                        break
                    t = tile_list[b]
                    ttile = sb3.tile([P, EMB], F32, tag="extab")
                    nc.sync.dma_start(out=ttile[:],
                                      in_=table_dram[t * P:(t + 1) * P, :])
                    selT = sb3.tile([P, P], F32, tag="selT")
                    nc.vector.tensor_tensor(out=selT[:], in0=iota_col[:],
                                            in1=bc[:, j * P:(j + 1) * P],
                                            op=ALU.is_equal)
                    pe = ps.tile([P, EMB], F32, space="PSUM", tag="w128")
                    nc.tensor.matmul(out=pe[:], lhsT=selT[:], rhs=ttile[:],
                                     start=True, stop=True)
                    so = sb3.tile([P, EMB], F32, tag="exout")
                    nc.scalar.activation(out=so[:], in_=pe[:], func=ACTF.Copy)
                    nc.sync.dma_start(out=out_dram[b * P:(b + 1) * P, :],
                                      in_=so[:])

        expand_stream(left_srel, B_LEFT, meta["left_tile"], netask_rm, o_left)
        expand_stream(rc_drel, B_RC, meta["rc_tile"], nevm_rm, o_rc)
        expand_stream(dep_drel_quad, B_DEP, meta["dep_tile"], netask_rm, o_rd)

        scope_out("edges")

    _split_multiwaits(nc)
    return nc


# ---------------------------------------------------------------- assembly

def _assemble(meta, results, asm):
    c = meta["cfg"]
    NT, NV, E1, E2 = c["NT"], c["NV"], c["E1"], c["E2"]
    SH = c["SH"]
    node = np.zeros((NT + NV, EMB), np.float32)
    for r in range(NCORE):
        node[r * SH:(r + 1) * SH] = results[r]["o_ne"]
    node[NT:] = results[0]["o_nevm"]

    edge = np.zeros((E1 + E2, 2 * EMB), np.float32)
    for r in range(NCORE):
        le = asm["left_eids"][r]
        lk = asm["left_kind"][r]
        lv = results[r]["o_left"]
        m = le >= 0
        ids = le[m].copy()
        ids[lk[m] == 1] += E1
        edge[ids, :EMB] = lv[m]
        rce = asm["rc_eids"][r]
        m = rce >= 0
        edge[rce[m], EMB:] = results[r]["o_rc"][m]
        de = asm["dep_eids"][r]
        m = de >= 0
        edge[de[m] + E1, EMB:] = results[r]["o_rd"][m]

    gsum = np.zeros(EMB, np.float32)
    for r in range(NCORE):
        gsum += results[r]["o_graph"][:, 0]
    gsum += results[0]["o_graph"][:, 1]
    graph = (gsum / (NT + NV)).reshape(1, EMB).astype(np.float32)
    return node, edge, graph


# ---------------------------------------------------------------- entry

def kernel(**inputs):
    cfg = CFG_FULL
    meta, per_core, asm = _prep(inputs, cfg)
    nc = _build(meta)
    res = run_bass_kernel_spmd(
        nc, per_core, core_ids=list(range(NCORE)),
        trace=bool(os.environ.get("KERNEL_TRACE")))
    if res.exec_time_ns is not None:
        print(f"HW exec time: {res.exec_time_ns} ns")
    return _assemble(meta, res.results, asm)


# revision 21
# speedup vs baseline: 1.3719x; 1.3719x over previous
"""Trainium2 Bass kernel for nn_BaseGinNetwork (GIN message passing).

Self-contained: host-side sharding prep (numpy) + one SPMD Bass/Tile program
run on 8 NeuronCores via bass_utils.run_bass_kernel_spmd.

Sharding:
- tasks row-sharded NT/8 per core; VM nodes replicated; compat edges
  src-sharded with a dense per-core count matrix M driving the VM aggregation
  as plain matmuls; partial VM aggregates merged through a concat-AllGather.
- dep edges dst-sharded; x[src] fetched by indirect-DMA gather from an
  AllGathered task table; scatter-add done as one-hot matmuls into PSUM.
- MLP chain runs feature-major so BN/bias/relu are per-partition ACT ops.
- edge_embeddings emitted as one-hot PE expansions; halves assembled on host.
"""
import contextlib
import copy as _copy
import dataclasses as _dc
import math
import os
import sys

import numpy as np

sys.path.insert(0, os.path.dirname(os.path.abspath(__file__)))

import concourse.bass as bass
import concourse.mybir as mybir
import concourse.tile as tile
from concourse.bass_utils import run_bass_kernel_spmd
from concourse.masks import make_identity
from concourse.vector_clock import ScopedClock as _ScopedClock

F32 = mybir.dt.float32
I32 = mybir.dt.int32
P = 128
NCORE = 8
HID, EMB = 256, 128

# ---------------------------------------------------------------- walrus fix
# This container's walrus encodes only ONE sync wait per instruction; Tile
# emits multi-wait instructions.  Split extra waits onto standalone wait-only
# EventSemaphore instructions placed just before, on the same engine.


def _patched_drain_and_barrier(self, tick_clock, wait_clock):
    nc = self.nc
    drain_inst = nc.sync.drain()
    wait_clock.add_sem_waits(
        drain_inst.ins, _ScopedClock({None: tick_clock.global_clock})
    )
    si = drain_inst.ins.sync_info
    ow = list(si.on_wait or []) if si is not None else []
    if len(ow) > 1:
        si.on_wait = ow[:1]
        drain_inst.ins.sync_info = si
        for w in ow[1:]:
            extra = nc.sync.drain()
            esi = extra.ins.sync_info
            if esi is None:
                esi = _dc.replace(si, on_wait=[w], on_update=[])
            else:
                esi.on_wait = [w]
                esi.on_update = []
            extra.ins.sync_info = esi
    nc.all_engine_barrier()
    assert self.sems is not None
    popped = nc._tile_sem_poison_stack.pop()
    assert popped is self._sem_poison
    nc.clear_and_free_semaphores(list(self.sems.allocated().values()))
    nc.all_engine_barrier()


def _split_multiwaits(nc):
    template = None
    for bb in nc.main_func.blocks:
        for ins in bb.instructions:
            if type(ins).__name__ == "InstEventSemaphore":
                template = ins
                break
        if template is not None:
            break
    assert template is not None
    counter = 0
    for bb in nc.main_func.blocks:
        insns = bb.instructions
        new_list = []
        for ins in insns:
            si = getattr(ins, "sync_info", None)
            ow = list(si.on_wait) if (si is not None and si.on_wait) else []
            if len(ow) > 1:
                for w in ow[:-1]:
                    ev = _copy.deepcopy(template)
                    ev.name = f"wsplit_{counter}"
                    counter += 1
                    ev.engine = ins.engine
                    esi = ev.sync_info
                    esi.on_wait = [w]
                    esi.on_update = []
                    ev.sync_info = esi
                    new_list.append(ev)
                si.on_wait = [ow[-1]]
                ins.sync_info = si
            new_list.append(ins)
        insns[:] = new_list


def _install_ntff_hook():
    import types

    try:
        from antenv.axon_hooks import get_axon_ntff_profile_hook  # noqa: F401

        return
    except ImportError:
        pass
    try:
        import antenv
        from trn_agent_boot.trn_boot import _ntff_profile_via_ctypes
    except ImportError:
        return
    mod = types.ModuleType("antenv.axon_hooks")
    state = {"hook": _ntff_profile_via_ctypes("/opt/axon/libaxon_pjrt.so")}
    mod.set_axon_ntff_profile_hook = lambda h: state.__setitem__("hook", h)
    mod.get_axon_ntff_profile_hook = lambda: state["hook"]
    sys.modules["antenv.axon_hooks"] = mod
    antenv.axon_hooks = mod


tile.TileContext._drain_and_barrier = _patched_drain_and_barrier
_install_ntff_hook()

# ---------------------------------------------------------------- config

CFG_FULL = dict(NT=50000, NV=1000, E1=200000, E2=100000)


def _derive(cfg):
    d = dict(cfg)
    NT, NV = cfg["NT"], cfg["NV"]
    d["SH"] = NT // NCORE
    d["SHT"] = math.ceil(d["SH"] / P)
    d["SHP"] = d["SHT"] * P
    d["NVT"] = math.ceil(NV / P)
    d["NVP"] = d["NVT"] * P
    d["SECTION"] = d["SHP"] + NV
    d["XTILES"] = math.ceil(NT / P)
    return d


# ---------------------------------------------------------------- host prep

def _prep(inputs, cfg):
    c = _derive(cfg)
    NT, NV, E1, E2 = c["NT"], c["NV"], c["E1"], c["E2"]
    SH, SHT, SHP, NVT, NVP, SECTION = (
        c["SH"], c["SHT"], c["SHP"], c["NVT"], c["NVP"], c["SECTION"])

    inp = {k: np.asarray(v) for k, v in inputs.items()}
    f32 = np.float32

    task_feats = np.stack([
        inp["task_state_scheduled"], inp["task_state_ready"],
        inp["task_length"], inp["task_completion_time"],
        inp["task_memory_req_mb"], inp["task_cpu_req_cores"],
    ], axis=0).astype(f32)
    vm_feats = np.stack([
        inp["vm_completion_time"], inp["vm_speed"], inp["vm_energy_rate"],
        inp["vm_memory_mb"], inp["vm_available_memory_mb"],
        inp["vm_used_memory_fraction"], inp["vm_active_tasks_count"],
        inp["vm_cpu_cores"], inp["vm_available_cpu_cores"],
        inp["vm_used_cpu_fraction_cores"],
    ], axis=0).astype(f32)

    # x-moment input, pre-permuted so device DMAs are contiguous:
    # chunk c holds 16 tiles; layout [c, p, t, f8] flattened to [c, p, 128].
    xaug = np.concatenate([task_feats.T, np.ones((NT, 1), f32),
                           np.zeros((NT, 1), f32)], axis=1)
    XT = c["XTILES"]
    xaug_t = np.zeros((XT * P, 8), f32)
    xaug_t[:NT] = xaug
    NCH = math.ceil(XT / 16)
    xmom = np.zeros((NCH, P, 16, 8), f32)
    for t in range(XT):
        ch, tt = divmod(t, 16)
        xmom[ch, :, tt, :] = xaug_t[t * P:(t + 1) * P]
    xmom = xmom.reshape(NCH, P, 128)

    x_own = np.zeros((NCORE, 8, SH), f32)
    for r in range(NCORE):
        x_own[r, :6] = task_feats[:, r * SH:(r + 1) * SH]
        x_own[r, 6] = 1.0

    vm_x = np.concatenate([vm_feats, np.ones((1, NV), f32),
                           np.zeros((1, NV), f32)], axis=0)
    wscale = np.ones((8, 1), f32); wscale[4, 0] = 1e-3
    vscale = np.ones((12, 1), f32); vscale[3, 0] = 1e-3; vscale[4, 0] = 1e-3
    vm_speed_row = vm_feats[1:2, :].copy()
    vm_cpu_row = vm_feats[7:8, :].copy()

    W = {}
    W["te_W1"] = np.concatenate([inp["te_W1"], inp["te_b1"][None, :],
                                 np.zeros((1, HID), f32)], axis=0).astype(f32)
    def rowpack(w):
        w = np.asarray(w, f32)
        k = w.shape[0] // P
        return np.concatenate([w[i * P:(i + 1) * P] for i in range(k)], axis=1)

    W["te_W2"] = rowpack(inp["te_W2"])
    W["te_W3"] = rowpack(inp["te_W3"])
    W["ve_W1"] = np.concatenate([inp["ve_W1"], inp["ve_b1"][None, :],
                                 np.zeros((1, HID), f32)], axis=0).astype(f32)
    W["ve_W2"] = rowpack(inp["ve_W2"])
    W["ve_W3"] = rowpack(inp["ve_W3"])
    W["g1_Wa"] = inp["g1_Wa"].astype(f32)
    W["g1_Wb"] = rowpack(inp["g1_Wb"])
    W["g2_Wc"] = rowpack(inp["g2_Wc"])
    W["g2_Wd"] = inp["g2_Wd"].astype(f32)

    def fmaj(v, nt):
        out = np.zeros((P, nt), f32)
        v = np.asarray(v, f32)
        for i in range(nt):
            seg = v[i * P:(i + 1) * P]
            out[:len(seg), i] = seg
        return out

    W["te_g1"] = fmaj(inp["te_g1"], 2);   W["te_be1"] = fmaj(inp["te_be1"], 2)
    W["te_g2"] = fmaj(inp["te_g2"], 2);   W["te_be2"] = fmaj(inp["te_be2"], 2)
    W["te_b3f"] = fmaj(inp["te_b3"], 1)
    W["ve_g1"] = fmaj(inp["ve_g1"], 2);   W["ve_be1"] = fmaj(inp["ve_be1"], 2)
    W["ve_g2"] = fmaj(inp["ve_g2"], 2);   W["ve_be2"] = fmaj(inp["ve_be2"], 2)
    W["ve_b3f"] = fmaj(inp["ve_b3"], 1)
    W["g1_baf"] = fmaj(inp["g1_ba"], 2);  W["g1_bbf"] = fmaj(inp["g1_bb"], 2)
    W["g2_bcf"] = fmaj(inp["g2_bc"], 1);  W["g2_bdf"] = fmaj(inp["g2_bd"], 1)

    csrc = np.asarray(inp["compat_src"], np.int64)
    cdst = np.asarray(inp["compat_dst"], np.int64)
    dsrc = np.asarray(inp["dep_src"], np.int64)
    ddst = np.asarray(inp["dep_dst"], np.int64)
    c_owner = csrc // SH
    d_owner = ddst // SH

    Mmat = np.zeros((NCORE, SHP, NVP), f32)
    for r in range(NCORE):
        m = c_owner == r
        np.add.at(Mmat[r], (csrc[m] - r * SH, cdst[m]), 1.0)

    # dep blocks: grouped by local dst tile, per-tile block counts padded to
    # the max over cores (SPMD-static structure).
    dep_e = [[[] for _ in range(SHT)] for _ in range(NCORE)]
    for i in range(E2):
        r = int(d_owner[i])
        dep_e[r][(int(ddst[i]) - r * SH) // P].append(i)
    dep_nb = [max(1, math.ceil(max(len(dep_e[r][t]) for r in range(NCORE)) / P))
              for t in range(SHT)]
    B_DEP = int(np.sum(dep_nb))
    dep_gidx = np.zeros((NCORE, B_DEP * P, 1), np.int32)
    dep_drel = np.full((NCORE, P, B_DEP), -1.0, f32)
    dep_tile = []
    dep_eids = np.full((NCORE, B_DEP * P), -1, np.int64)
    b = 0
    for t in range(SHT):
        for k in range(dep_nb[t]):
            dep_tile.append(t)
            for r in range(NCORE):
                ids = dep_e[r][t][k * P:(k + 1) * P]
                for j, eid in enumerate(ids):
                    s = int(dsrc[eid])
                    dep_gidx[r, b * P + j, 0] = (s // SH) * SECTION + (s % SH)
                    dep_drel[r, j, b] = (int(ddst[eid]) - r * SH) - t * P
                    dep_eids[r, b * P + j] = eid
            b += 1
    assert b == B_DEP

    # edge-left stream: all edges grouped by local src tile of their owner.
    left_e = [[[] for _ in range(SHT)] for _ in range(NCORE)]
    for i in range(E1):
        r = int(c_owner[i])
        left_e[r][(int(csrc[i]) - r * SH) // P].append((0, i))
    for i in range(E2):
        r = int(dsrc[i] // SH)
        left_e[r][(int(dsrc[i]) - r * SH) // P].append((1, i))
    left_nb = [max(1, math.ceil(max(len(left_e[r][t]) for r in range(NCORE)) / P))
               for t in range(SHT)]
    B_LEFT = int(np.sum(left_nb))
    NQ_L = math.ceil(B_LEFT / 4)
    left_srel = np.full((NCORE, NQ_L, 512), -1.0, f32)
    left_tile = []
    left_eids = np.full((NCORE, B_LEFT * P), -1, np.int64)
    left_kind = np.zeros((NCORE, B_LEFT * P), np.int8)
    b = 0
    for t in range(SHT):
        for k in range(left_nb[t]):
            left_tile.append(t)
            for r in range(NCORE):
                ids = left_e[r][t][k * P:(k + 1) * P]
                for j, (kind, eid) in enumerate(ids):
                    s = int(csrc[eid] if kind == 0 else dsrc[eid])
                    left_srel[r, b // 4, (b % 4) * P + j] = (s % SH) - t * P
                    left_eids[r, b * P + j] = eid
                    left_kind[r, b * P + j] = kind
            b += 1
    assert b == B_LEFT

    # edge-right compat: compat edges (src owner) grouped by vm dst tile.
    rc_e = [[[] for _ in range(NVT)] for _ in range(NCORE)]
    for i in range(E1):
        rc_e[int(c_owner[i])][int(cdst[i]) // P].append(i)
    rc_nb = [max(1, math.ceil(max(len(rc_e[r][t]) for r in range(NCORE)) / P))
             for t in range(NVT)]
    B_RC = int(np.sum(rc_nb))
    NQ_RC = math.ceil(B_RC / 4)
    rc_drel = np.full((NCORE, NQ_RC, 512), -1.0, f32)
    rc_tile = []
    rc_eids = np.full((NCORE, B_RC * P), -1, np.int64)
    b = 0
    for t in range(NVT):
        for k in range(rc_nb[t]):
            rc_tile.append(t)
            for r in range(NCORE):
                ids = rc_e[r][t][k * P:(k + 1) * P]
                for j, eid in enumerate(ids):
                    rc_drel[r, b // 4, (b % 4) * P + j] = int(cdst[eid]) - t * P
                    rc_eids[r, b * P + j] = eid
            b += 1
    assert b == B_RC

    # edge-right dep: reuse dep blocks; quad layout of dep_drel for selT.
    NQ_D = math.ceil(B_DEP / 4)
    dep_drel_quad = np.full((NCORE, NQ_D, 512), -1.0, f32)
    for b in range(B_DEP):
        dep_drel_quad[:, b // 4, (b % 4) * P:(b % 4) * P + P] = dep_drel[:, :, b]

    def quad_segs(tile_list, nb):
        segs = []
        for q in range(math.ceil(nb / 4)):
            blks = tile_list[q * 4:q * 4 + 4]
            s = []
            for j, t in enumerate(blks):
                if s and s[-1][0] == t:
                    s[-1] = (t, s[-1][1], s[-1][2] + 1)
                else:
                    s.append((t, j, 1))
            segs.append(s)
        return segs

    meta = dict(cfg=c, B_DEP=B_DEP, dep_tile=dep_tile, B_LEFT=B_LEFT,
                left_tile=left_tile, B_RC=B_RC, rc_tile=rc_tile, NCH=NCH,
                left_segs=quad_segs(left_tile, B_LEFT),
                rc_segs=quad_segs(rc_tile, B_RC),
                dep_segs=quad_segs(dep_tile, B_DEP))

    per_core = []
    for r in range(NCORE):
        d = dict(
            xmom=xmom, x_own=x_own[r], vm_x=vm_x, M=Mmat[r],
            wscale=wscale, vscale=vscale, vm_speed_row=vm_speed_row,
            vm_cpu_row=vm_cpu_row,
            dep_gidx=dep_gidx[r], dep_drel=dep_drel[r],
            dep_drel_quad=dep_drel_quad[r], left_srel=left_srel[r],
            rc_drel=rc_drel[r])
        d.update({k: np.ascontiguousarray(v) for k, v in W.items()})
        per_core.append(d)

    asm = dict(meta=meta, dep_eids=dep_eids, left_eids=left_eids,
               left_kind=left_kind, rc_eids=rc_eids)
    return meta, per_core, asm


# ---------------------------------------------------------------- device

def _build(meta):
    c = meta["cfg"]
    NT, NV = c["NT"], c["NV"]
    SH, SHT, SHP, NVT, NVP, SECTION = (
        c["SH"], c["SHT"], c["SHP"], c["NVT"], c["NVP"], c["SECTION"])
    B_DEP, B_LEFT, B_RC = meta["B_DEP"], meta["B_LEFT"], meta["B_RC"]
    NCH = meta["NCH"]
    XT = c["XTILES"]
    NBL = math.ceil(SH / 512)
    # vm column chunks (matmul N <= 512)
    VCH = [(i, min(NV, i + 512)) for i in range(0, NV, 512)]

    nc = bass.Bass("TRN2", target_bir_lowering=False, debug=False)

    def ein(name, shape, dtype=F32):
        return nc.dram_tensor(name, shape, dtype, kind="ExternalInput")

    xmom_d = ein("xmom", [NCH, P, 128])
    x_own = ein("x_own", [8, SH])
    vm_x = ein("vm_x", [12, NV])
    wscale = ein("wscale", [8, 1])
    vscale = ein("vscale", [12, 1])
    vm_speed_row = ein("vm_speed_row", [1, NV])
    vm_cpu_row = ein("vm_cpu_row", [1, NV])
    M_in = ein("M", [SHP, NVP])
    dep_gidx = ein("dep_gidx", [B_DEP * P, 1], I32)
    dep_drel = ein("dep_drel", [P, B_DEP])
    dep_drel_quad = ein("dep_drel_quad", [math.ceil(B_DEP / 4), 512])
    left_srel = ein("left_srel", [math.ceil(B_LEFT / 4), 512])
    rc_drel = ein("rc_drel", [math.ceil(B_RC / 4), 512])

    te_W1 = ein("te_W1", [8, HID]);    te_W2 = ein("te_W2", [P, 2 * HID])
    te_W3 = ein("te_W3", [P, 2 * EMB])
    ve_W1 = ein("ve_W1", [12, HID]);   ve_W2 = ein("ve_W2", [P, 2 * HID])
    ve_W3 = ein("ve_W3", [P, 2 * EMB])
    te_g1 = ein("te_g1", [P, 2]);      te_be1 = ein("te_be1", [P, 2])
    te_g2 = ein("te_g2", [P, 2]);      te_be2 = ein("te_be2", [P, 2])
    te_b3f = ein("te_b3f", [P, 1])
    ve_g1 = ein("ve_g1", [P, 2]);      ve_be1 = ein("ve_be1", [P, 2])
    ve_g2 = ein("ve_g2", [P, 2]);      ve_be2 = ein("ve_be2", [P, 2])
    ve_b3f = ein("ve_b3f", [P, 1])
    g1_Wa = ein("g1_Wa", [EMB, HID]);  g1_Wb = ein("g1_Wb", [P, 2 * HID])
    g2_Wc = ein("g2_Wc", [P, 2 * EMB]);  g2_Wd = ein("g2_Wd", [EMB, EMB])
    g1_baf = ein("g1_baf", [P, 2]);    g1_bbf = ein("g1_bbf", [P, 2])
    g2_bcf = ein("g2_bcf", [P, 1]);    g2_bdf = ein("g2_bdf", [P, 1])

    def eout(name, shape, dtype=F32):
        return nc.dram_tensor(name, shape, dtype, kind="ExternalOutput")

    o_ne = eout("o_ne", [SH, EMB])
    o_nevm = eout("o_nevm", [NV, EMB])
    o_graph = eout("o_graph", [P, 2])
    o_left = eout("o_left", [P, B_LEFT * P])
    o_rc = eout("o_rc", [P, B_RC * P])
    o_rd = eout("o_rd", [P, B_DEP * P])

    cc1_in = nc.dram_tensor("cc1_in", [SECTION, EMB], F32)
    cc1_out = nc.dram_tensor("cc1_out", [NCORE * SECTION, EMB], F32)
    cc2_in = nc.dram_tensor("cc2_in", [SECTION, EMB], F32)
    cc2_out = nc.dram_tensor("cc2_out", [NCORE * SECTION, EMB], F32)
    st_in = nc.dram_tensor("st_in", [P, 4], F32)
    st_out = nc.dram_tensor("st_out", [P, 4], F32)
    netask_rm = nc.dram_tensor("netask_rm", [SHP, EMB], F32)
    p2_d = nc.dram_tensor("p2_d", [P, 2 * SH], F32)
    th_fm = nc.dram_tensor("th_fm", [P, SH], F32)
    y2_fm = nc.dram_tensor("y2_fm", [P, SH], F32)
    nevm_rm = nc.dram_tensor("nevm_rm", [NVP, EMB], F32)

    RG = [list(range(NCORE))]
    AX = mybir.AxisListType.X
    ALU = mybir.AluOpType
    ACTF = mybir.ActivationFunctionType

    with tile.TileContext(nc) as tc, contextlib.ExitStack() as ctx:
        const = ctx.enter_context(tc.tile_pool(name="const", bufs=1))
        sb = ctx.enter_context(tc.tile_pool(name="sb", bufs=2))
        sb3 = ctx.enter_context(tc.tile_pool(name="sb3", bufs=3))
        ps = ctx.enter_context(tc.tile_pool(name="ps", bufs=2, space="PSUM"))
        ps_acc = ctx.enter_context(
            tc.tile_pool(name="ps_acc", bufs=2, space="PSUM"))
        ps_vg = ctx.enter_context(
            tc.tile_pool(name="ps_vg", bufs=1, space="PSUM"))

        _scope_ids = {}

        def scope_in(name):
            _scope_ids[name] = nc.enter_named_scope(name, False)[0]

        def scope_out(name):
            nc.leave_named_scope(name, _scope_ids.pop(name), False)

        # -------- constants
        ident = const.tile([P, P], F32, tag="ident")
        make_identity(nc, ident[:])
        iota_row_i = const.tile([P, P], I32, tag="ioti")
        nc.gpsimd.iota(iota_row_i[:], pattern=[[1, P]], channel_multiplier=0)
        iota_row = const.tile([P, P], F32, tag="iotr")
        nc.vector.tensor_copy(out=iota_row[:], in_=iota_row_i[:])
        iota_col_i = const.tile([P, P], I32, tag="iotci")
        nc.gpsimd.iota(iota_col_i[:], pattern=[[0, P]], channel_multiplier=1)
        iota_col = const.tile([P, P], F32, tag="iotc")
        nc.vector.tensor_copy(out=iota_col[:], in_=iota_col_i[:])
        ones_row = const.tile([1, 512], F32, tag="ones")
        nc.gpsimd.memset(ones_row[:], 1.0)
        ones8 = const.tile([8, 1], F32, tag="ones8")
        nc.gpsimd.memset(ones8[:], 1.0)

        def load(t, tag):
            tl = const.tile(list(t.shape), t.dtype, tag=tag)
            nc.sync.dma_start(out=tl[:], in_=t[:])
            return tl

        w_te1 = load(te_W1, "wte1"); w_te2 = load(te_W2, "wte2")
        w_te3 = load(te_W3, "wte3")
        w_ve1 = load(ve_W1, "wve1"); w_ve2 = load(ve_W2, "wve2")
        w_ve3 = load(ve_W3, "wve3")
        w_a = load(g1_Wa, "wa"); w_b = load(g1_Wb, "wb")
        w_c = load(g2_Wc, "wc"); w_d = load(g2_Wd, "wd")
        v_te_g1 = load(te_g1, "vg1"); v_te_be1 = load(te_be1, "vb1")
        v_te_g2 = load(te_g2, "vg2"); v_te_be2 = load(te_be2, "vb2")
        v_te_b3 = load(te_b3f, "vb3")
        v_ve_g1 = load(ve_g1, "wg1"); v_ve_be1 = load(ve_be1, "wb1")
        v_ve_g2 = load(ve_g2, "wg2"); v_ve_be2 = load(ve_be2, "wb2")
        v_ve_b3 = load(ve_b3f, "wb3v")
        v_ba = load(g1_baf, "vba"); v_bb = load(g1_bbf, "vbb")
        v_bc = load(g2_bcf, "vbc"); v_bd = load(g2_bdf, "vbd")

        scope_in("pre")
        # -------- vm input transform + maxc (offset-0 partition ops only;
        # partition placement done via DMA)
        vmx_raw = const.tile([12, NV], F32, tag="vmxr")
        nc.sync.dma_start(out=vmx_raw[:], in_=vm_x[:])
        vcpu = const.tile([1, NV], F32, tag="vcpu")
        nc.sync.dma_start(out=vcpu[:], in_=vm_cpu_row[:])
        maxc = const.tile([1, 1], F32, tag="maxc")
        nc.vector.reduce_max(out=maxc[:], in_=vcpu[:], axis=AX)
        maxc1 = const.tile([1, 1], F32, tag="maxc1")
        nc.vector.tensor_scalar_max(maxc1[:], maxc[:], 1.0)
        rmaxc = const.tile([1, 1], F32, tag="rmaxc")
        nc.vector.reciprocal(out=rmaxc[:], in_=maxc1[:])
        rm8 = const.tile([8, 1], F32, tag="rm8")
        nc.gpsimd.memset(rm8[:], 1.0)
        nc.sync.dma_start(out=rm8[5:6, :], in_=rmaxc[0:1, 0:1])
        wsc = const.tile([8, 1], F32, tag="wsc")
        nc.sync.dma_start(out=wsc[:], in_=wscale[:])
        nc.vector.tensor_tensor(out=wsc[:], in0=wsc[:], in1=rm8[:], op=ALU.mult)
        w_te1s = const.tile([8, HID], F32, tag="wte1s")
        nc.vector.tensor_scalar_mul(w_te1s[:], w_te1[:], wsc[:, 0:1])
        rm12 = const.tile([12, 1], F32, tag="rm12")
        nc.gpsimd.memset(rm12[:], 1.0)
        nc.sync.dma_start(out=rm12[7:8, :], in_=rmaxc[0:1, 0:1])
        nc.sync.dma_start(out=rm12[8:9, :], in_=rmaxc[0:1, 0:1])
        vsc = const.tile([12, 1], F32, tag="vsc")
        nc.sync.dma_start(out=vsc[:], in_=vscale[:])
        nc.vector.tensor_tensor(out=vsc[:], in0=vsc[:], in1=rm12[:], op=ALU.mult)
        vmx = const.tile([12, NV], F32, tag="vmx")
        nc.vector.tensor_scalar_mul(vmx[:], vmx_raw[:], vsc[:, 0:1])
        spd = const.tile([1, NV], F32, tag="spd")
        nc.sync.dma_start(out=spd[:], in_=vm_speed_row[:])
        nc.vector.tensor_scalar_add(spd[:], spd[:], 1e-8)
        nc.vector.reciprocal(out=spd[:], in_=spd[:])
        nc.sync.dma_start(out=vmx[1:2, :], in_=spd[0:1, :])

        # -------- x moments: Caug = sum over rows of [x | 1] outer products
        cmom_ps = ps_acc.tile([8, 8], F32, space="PSUM", tag="acc")
        for ch in range(NCH):
            xm = sb3.tile([P, 128], F32, tag="xm")
            nc.sync.dma_start(out=xm[:], in_=xmom_d[ch])
            for t in range(16):
                gt = ch * 16 + t
                if gt >= XT:
                    break
                nc.tensor.matmul(out=cmom_ps[:], lhsT=xm[:, t * 8:t * 8 + 8],
                                 rhs=xm[:, t * 8:t * 8 + 8],
                                 start=(gt == 0), stop=(gt == XT - 1))
        caug = const.tile([8, 8], F32, tag="caug")
        nc.vector.tensor_copy(out=caug[:], in_=cmom_ps[:])

        # BN1 scale/bias from moments (scaled W1')
        cw_ps = ps.tile([8, HID], F32, space="PSUM", tag="w512")
        nc.tensor.matmul(out=cw_ps[:], lhsT=caug[:], rhs=w_te1s[:],
                         start=True, stop=True)
        ep = sb.tile([8, HID], F32, tag="ep")
        nc.vector.tensor_tensor(out=ep[:], in0=w_te1s[:], in1=cw_ps[:],
                                op=ALU.mult)
        bn1_s = const.tile([P, 2], F32, tag="bn1s")
        bn1_b = const.tile([P, 2], F32, tag="bn1b")
        ep2 = sb.tile([P, 2], F32, tag="ep2")
        mean1 = sb.tile([P, 2], F32, tag="mean1")
        for f in range(2):
            pp = ps.tile([P, 1], F32, space="PSUM", tag="w128")
            nc.tensor.matmul(out=pp[:], lhsT=ep[:, f * P:(f + 1) * P],
                             rhs=ones8[:], start=True, stop=True)
            nc.scalar.activation(out=ep2[:, f:f + 1], in_=pp[:],
                                 func=ACTF.Copy, scale=1.0 / NT)
            pp2 = ps.tile([P, 1], F32, space="PSUM", tag="w128")
            nc.tensor.matmul(out=pp2[:], lhsT=w_te1s[:, f * P:(f + 1) * P],
                             rhs=caug[:, 6:7], start=True, stop=True)
            nc.scalar.activation(out=mean1[:, f:f + 1], in_=pp2[:],
                                 func=ACTF.Copy, scale=1.0 / NT)
        var1 = sb.tile([P, 2], F32, tag="var1")
        nc.vector.tensor_tensor(out=var1[:], in0=mean1[:], in1=mean1[:],
                                op=ALU.mult)
        nc.vector.tensor_tensor(out=var1[:], in0=ep2[:], in1=var1[:],
                                op=ALU.subtract)
        nc.vector.tensor_scalar_add(var1[:], var1[:], 1e-5)
        nc.scalar.activation(out=var1[:], in_=var1[:], func=ACTF.Sqrt)
        nc.vector.reciprocal(out=var1[:], in_=var1[:])
        nc.vector.tensor_tensor(out=bn1_s[:], in0=v_te_g1[:], in1=var1[:],
                                op=ALU.mult)
        nc.vector.tensor_tensor(out=bn1_b[:], in0=mean1[:], in1=bn1_s[:],
                                op=ALU.mult)
        nc.vector.tensor_tensor(out=bn1_b[:], in0=v_te_be1[:], in1=bn1_b[:],
                                op=ALU.subtract)

        scope_out("pre")
        scope_in("enc")
        # -------- task encoder to p2 (raw), with BN2 stat accumulation
        stats = const.tile([P, 4], F32, tag="stats")
        nc.gpsimd.memset(stats[:], 0.0)
        for nb in range(NBL):
            n0 = nb * 512
            n1 = min(SH, n0 + 512)
            w = n1 - n0
            xob = sb.tile([8, 512], F32, tag="xob")
            nc.sync.dma_start(out=xob[:, :w], in_=x_own[:, n0:n1])
            h1b = sb.tile([P, 2 * 512], F32, tag="h1b")
            for f in range(2):
                pp = ps.tile([P, 512], F32, space="PSUM", tag="w512")
                nc.tensor.matmul(out=pp[:, :w],
                                 lhsT=w_te1s[:, f * P:(f + 1) * P],
                                 rhs=xob[:, :w], start=True, stop=True)
                nc.scalar.activation(out=h1b[:, f * 512:f * 512 + w],
                                     in_=pp[:, :w], func=ACTF.Relu,
                                     bias=bn1_b[:, f:f + 1],
                                     scale=bn1_s[:, f:f + 1])
            for f in range(2):
                pp = ps.tile([P, 512], F32, space="PSUM", tag="w512")
                for k in range(2):
                    nc.tensor.matmul(
                        out=pp[:, :w],
                        lhsT=w_te2[:, k * HID + f * P:k * HID + (f + 1) * P],
                        rhs=h1b[:, k * 512:k * 512 + w],
                        start=(k == 0), stop=(k == 1))
                p2b = sb.tile([P, 512], F32, tag="p2b")
                nc.vector.tensor_copy(out=p2b[:, :w], in_=pp[:, :w])
                nc.sync.dma_start(out=p2_d[:, f * SH + n0:f * SH + n1],
                                  in_=p2b[:, :w])
                r1 = sb.tile([P, 1], F32, tag="str1")
                nc.vector.reduce_sum(out=r1[:], in_=p2b[:, :w], axis=AX)
                nc.vector.tensor_add(out=stats[:, f:f + 1],
                                     in0=stats[:, f:f + 1], in1=r1[:])
                sqb = sb.tile([P, 512], F32, tag="sqb")
                nc.vector.tensor_tensor(out=sqb[:, :w], in0=p2b[:, :w],
                                        in1=p2b[:, :w], op=ALU.mult)
                r2 = sb.tile([P, 1], F32, tag="str1")
                nc.vector.reduce_sum(out=r2[:], in_=sqb[:, :w], axis=AX)
                nc.vector.tensor_add(out=stats[:, 2 + f:3 + f],
                                     in0=stats[:, 2 + f:3 + f], in1=r2[:])

        scope_out("enc")
        scope_in("s1_ar")
        # -------- S1: stats AllReduce
        nc.sync.dma_start(out=st_in[:], in_=stats[:])
        nc.gpsimd.collective_compute("AllReduce", ALU.add, replica_groups=RG,
                                     ins=[st_in[:]], outs=[st_out[:]])
        st_sb = sb.tile([P, 4], F32, tag="stsb")
        nc.sync.dma_start(out=st_sb[:], in_=st_out[:])
        bn2_s = const.tile([P, 2], F32, tag="bn2s")
        bn2_b = const.tile([P, 2], F32, tag="bn2b")
        mean2 = sb.tile([P, 2], F32, tag="mean2")
        var2 = sb.tile([P, 2], F32, tag="var2")
        nc.scalar.activation(out=mean2[:], in_=st_sb[:, 0:2], func=ACTF.Copy,
                             scale=1.0 / NT)
        nc.scalar.activation(out=var2[:], in_=st_sb[:, 2:4], func=ACTF.Copy,
                             scale=1.0 / NT)
        m2sq = sb.tile([P, 2], F32, tag="m2sq")
        nc.vector.tensor_tensor(out=m2sq[:], in0=mean2[:], in1=mean2[:],
                                op=ALU.mult)
        nc.vector.tensor_tensor(out=var2[:], in0=var2[:], in1=m2sq[:],
                                op=ALU.subtract)
        nc.vector.tensor_scalar_add(var2[:], var2[:], 1e-5)
        nc.scalar.activation(out=var2[:], in_=var2[:], func=ACTF.Sqrt)
        nc.vector.reciprocal(out=var2[:], in_=var2[:])
        nc.vector.tensor_tensor(out=bn2_s[:], in0=v_te_g2[:], in1=var2[:],
                                op=ALU.mult)
        nc.vector.tensor_tensor(out=bn2_b[:], in0=mean2[:], in1=bn2_s[:],
                                op=ALU.mult)
        nc.vector.tensor_tensor(out=bn2_b[:], in0=v_te_be2[:], in1=bn2_b[:],
                                op=ALU.subtract)

        # transpose helper: f-major SBUF block [128, <=512] -> row-major
        # DRAM rows (one or two destinations)
        def transpose_block(drams, blk, n0, w):
            for k in range(math.ceil(w / P)):
                c0 = k * P
                c1 = min(w, c0 + P)
                w2 = c1 - c0
                tp = ps.tile([P, P], F32, space="PSUM", tag="w128")
                nc.tensor.transpose(out=tp[:w2, :], in_=blk[:, c0:c1],
                                    identity=ident[:])
                so = sb3.tile([P, P], F32, tag="tpo")
                nc.scalar.activation(out=so[:w2, :], in_=tp[:w2, :],
                                     func=ACTF.Copy)
                for dram, row0 in drams:
                    nc.sync.dma_start(
                        out=dram[row0 + n0 + c0:row0 + n0 + c1, :],
                        in_=so[:w2, :])

        scope_out("s1_ar")
        scope_in("enc2")
        # -------- h2 = relu(bn2(p2)); task_h = W3^T h2 + b3 (f-major)
        for nb in range(NBL):
            n0 = nb * 512
            n1 = min(SH, n0 + 512)
            w = n1 - n0
            h2b = sb.tile([P, 2 * 512], F32, tag="h1b")
            for f in range(2):
                p2l = sb.tile([P, 512], F32, tag="p2b")
                nc.sync.dma_start(out=p2l[:, :w],
                                  in_=p2_d[:, f * SH + n0:f * SH + n1])
                nc.scalar.activation(out=h2b[:, f * 512:f * 512 + w],
                                     in_=p2l[:, :w],
                                     func=ACTF.Relu, bias=bn2_b[:, f:f + 1],
                                     scale=bn2_s[:, f:f + 1])
            pp = ps.tile([P, 512], F32, space="PSUM", tag="w512")
            for k in range(2):
                nc.tensor.matmul(out=pp[:, :w], lhsT=w_te3[:, k * EMB:(k + 1) * EMB],
                                 rhs=h2b[:, k * 512:k * 512 + w],
                                 start=(k == 0), stop=(k == 1))
            thb = sb.tile([P, 512], F32, tag="thb")
            nc.vector.tensor_tensor(
                out=thb[:, :w], in0=pp[:, :w],
                in1=v_te_b3[:, 0:1].to_broadcast([P, w]), op=ALU.add)
            nc.sync.dma_start(out=th_fm[:, n0:n1], in_=thb[:, :w])
            transpose_block([(cc1_in, 0)], thb, n0, w)
        if SHP > SH:
            zpad = sb.tile([SHP - SH, EMB], F32, tag="zpad")
            nc.gpsimd.memset(zpad[:], 0.0)
            nc.sync.dma_start(out=cc1_in[SH:SHP, :], in_=zpad[:])

        scope_out("enc2")
        scope_in("vmenc")
        # -------- vm encoder (replicated, local BN)
        def bn_local(praw, dst, f, n, g, be, relu):
            s1 = sb.tile([P, 1], F32, tag="vmst_s1")
            nc.vector.reduce_sum(out=s1[:], in_=praw[:, :n], axis=AX)
            sqv = sb.tile([P, NV], F32, tag="vmsq")
            nc.vector.tensor_tensor(out=sqv[:, :n], in0=praw[:, :n],
                                    in1=praw[:, :n], op=ALU.mult)
            s2 = sb.tile([P, 1], F32, tag="vmst_s2")
            nc.vector.reduce_sum(out=s2[:], in_=sqv[:, :n], axis=AX)
            mu = sb.tile([P, 1], F32, tag="vmst_mu")
            nc.scalar.activation(out=mu[:], in_=s1[:], func=ACTF.Copy,
                                 scale=1.0 / n)
            e2 = sb.tile([P, 1], F32, tag="vmst_e2")
            nc.scalar.activation(out=e2[:], in_=s2[:], func=ACTF.Copy,
                                 scale=1.0 / n)
            vv = sb.tile([P, 1], F32, tag="vmst_vv")
            nc.vector.tensor_tensor(out=vv[:], in0=mu[:], in1=mu[:], op=ALU.mult)
            nc.vector.tensor_tensor(out=vv[:], in0=e2[:], in1=vv[:],
                                    op=ALU.subtract)
            nc.vector.tensor_scalar_add(vv[:], vv[:], 1e-5)
            nc.scalar.activation(out=vv[:], in_=vv[:], func=ACTF.Sqrt)
            nc.vector.reciprocal(out=vv[:], in_=vv[:])
            sc = sb.tile([P, 1], F32, tag="vmst_sc")
            nc.vector.tensor_tensor(out=sc[:], in0=g[:, f:f + 1], in1=vv[:],
                                    op=ALU.mult)
            bi = sb.tile([P, 1], F32, tag="vmst_bi")
            nc.vector.tensor_tensor(out=bi[:], in0=mu[:], in1=sc[:], op=ALU.mult)
            nc.vector.tensor_tensor(out=bi[:], in0=be[:, f:f + 1], in1=bi[:],
                                    op=ALU.subtract)
            nc.scalar.activation(out=dst, in_=praw[:, :n],
                                 func=ACTF.Relu if relu else ACTF.Copy,
                                 bias=bi[:, 0:1], scale=sc[:, 0:1])

        def mm_chunks(out_sb, lhsT_list, rhs_sb, rhs_off, n):
            """psum-chunked matmul: out_sb[:, :n] = sum_k lhsT_k^T rhs_k."""
            for (c0, c1) in [(i, min(n, i + 512)) for i in range(0, n, 512)]:
                pp = ps.tile([P, 512], F32, space="PSUM", tag="w512")
                for ki, (lh, rh) in enumerate(zip(lhsT_list, rhs_off)):
                    nc.tensor.matmul(out=pp[:, :c1 - c0], lhsT=lh,
                                     rhs=rhs_sb[:, rh + c0:rh + c1],
                                     start=(ki == 0),
                                     stop=(ki == len(lhsT_list) - 1))
                yield (c0, c1, pp)

        vp1raw = const.tile([P, 2 * NV], F32, tag="vmX0")
        for f in range(2):
            for c0, c1, pp in mm_chunks(None, [w_ve1[:, f * P:(f + 1) * P]],
                                        vmx, [0], NV):
                nc.vector.tensor_copy(out=vp1raw[:, f * NV + c0:f * NV + c1],
                                      in_=pp[:, :c1 - c0])
        vh1 = const.tile([P, 2 * NV], F32, tag="vmB")
        for f in range(2):
            bn_local(vp1raw[:, f * NV:(f + 1) * NV], vh1[:, f * NV:(f + 1) * NV],
                     f, NV, v_ve_g1, v_ve_be1, True)
        vp2raw = const.tile([P, 2 * NV], F32, tag="vmX0")
        for f in range(2):
            for c0, c1, pp in mm_chunks(
                    None,
                    [w_ve2[:, k * HID + f * P:k * HID + (f + 1) * P] for k in range(2)],
                    vh1, [0, NV], NV):
                nc.vector.tensor_copy(out=vp2raw[:, f * NV + c0:f * NV + c1],
                                      in_=pp[:, :c1 - c0])
        vh2 = const.tile([P, 2 * NV], F32, tag="vmC")
        for f in range(2):
            bn_local(vp2raw[:, f * NV:(f + 1) * NV], vh2[:, f * NV:(f + 1) * NV],
                     f, NV, v_ve_g2, v_ve_be2, True)
        vmh_t = const.tile([P, NV], F32, tag="vmh")
        for c0, c1, pp in mm_chunks(
                None, [w_ve3[:, k * EMB:(k + 1) * EMB] for k in range(2)],
                vh2, [0, NV], NV):
            nc.vector.tensor_tensor(
                out=vmh_t[:, c0:c1], in0=pp[:, :c1 - c0],
                in1=v_ve_b3[:, 0:1].to_broadcast([P, c1 - c0]), op=ALU.add)

        scope_out("vmenc")
        scope_in("compat1")
        # -------- compat partial agg via count matrix M
        def compat_agg(table_dram, out_cc):
            # f-major: out[f, v] = sum_n table[n, f] * M[n, v] with N=512-wide
            # rhs (4 vm tiles per matmul).  Accumulation per psum bank is
            # contiguous (g outer, t inner).
            NVG = math.ceil(NVP / 512)
            for g in range(NVG):
                wv = min(512, NVP - g * 512)
                pv_t = ps_vg.tile([P, 512], F32, space="PSUM",
                                  tag=f"vg{g % 2}")
                for t in range(SHT):
                    ttile = sb3.tile([P, EMB], F32, tag="cttile")
                    nc.sync.dma_start(out=ttile[:],
                                      in_=table_dram[t * P:(t + 1) * P, :])
                    mwide = sb.tile([P, 512], F32, tag="mwide")
                    nc.sync.dma_start(
                        out=mwide[:, :wv],
                        in_=M_in[t * P:(t + 1) * P, g * 512:g * 512 + wv])
                    nc.tensor.matmul(out=pv_t[:, :wv], lhsT=ttile[:],
                                     rhs=mwide[:, :wv],
                                     start=(t == 0), stop=(t == SHT - 1))
                pvc = sb.tile([P, 512], F32, tag="pvc")
                nc.scalar.activation(out=pvc[:, :wv], in_=pv_t[:, :wv],
                                     func=ACTF.Copy)
                for vt in range(g * 4, min(NVT, g * 4 + 4)):
                    go = vt - g * 4
                    tp = ps.tile([P, P], F32, space="PSUM", tag="w128")
                    nc.tensor.transpose(out=tp[:],
                                        in_=pvc[:, go * P:(go + 1) * P],
                                        identity=ident[:])
                    so = sb3.tile([P, EMB], F32, tag="vaggout")
                    nc.scalar.activation(out=so[:], in_=tp[:], func=ACTF.Copy)
                    r0 = SHP + vt * P
                    r1 = min(SHP + NV, r0 + P)
                    if r1 > r0:
                        nc.sync.dma_start(out=out_cc[r0:r1, :],
                                          in_=so[:r1 - r0, :])

        compat_agg(cc1_in, cc1_in)

        scope_out("compat1")
        scope_in("s2_ag")
        # -------- S2: AllGather (task_h || vm partials)
        nc.gpsimd.collective_compute("AllGather", ALU.bypass, replica_groups=RG,
                                     ins=[cc1_in[:]], outs=[cc1_out[:]])

        def vm_agg_reduce(cc_out_t, tag):
            acc = const.tile([P, NVT * EMB], F32, tag=tag)
            for r in range(NCORE):
                part = sb.tile([P, NVT * EMB], F32, tag="vmaggld")
                for vt in range(NVT):
                    r0 = r * SECTION + SHP + vt * P
                    r1 = min(r * SECTION + SHP + NV, r0 + P)
                    if r1 <= r0:
                        continue
                    nc.sync.dma_start(
                        out=part[:r1 - r0, vt * EMB:(vt + 1) * EMB],
                        in_=cc_out_t[r0:r1, :])
                if r == 0:
                    nc.vector.tensor_copy(out=acc[:], in_=part[:])
                else:
                    nc.vector.tensor_add(out=acc[:], in0=acc[:], in1=part[:])
            agg_t = const.tile([P, NVT * P], F32, tag=tag + "t")
            for vt in range(NVT):
                tp = ps.tile([P, P], F32, space="PSUM", tag="w128")
                nc.tensor.transpose(out=tp[:], in_=acc[:, vt * EMB:(vt + 1) * EMB],
                                    identity=ident[:])
                nc.scalar.activation(out=agg_t[:, vt * P:(vt + 1) * P],
                                     in_=tp[:], func=ACTF.Copy)
            return agg_t

        vm_agg1_t = vm_agg_reduce(cc1_out, "vagg")

        scope_out("s2_ag")
        scope_in("layer1")
        # -------- dep aggregation + fused MLP consumption
        dep_tile_list = meta["dep_tile"]
        blocks_of_tile = [[] for _ in range(SHT)]
        for b, t in enumerate(dep_tile_list):
            blocks_of_tile[t].append(b)
        drel_sb = const.tile([P, B_DEP], F32, tag="drel")
        nc.sync.dma_start(out=drel_sb[:], in_=dep_drel[:])

        def dep_agg_consume(cc_out_t, x_fm, consume):
            for nb2 in range(NBL):
                n0 = nb2 * 512
                n1 = min(SH, n0 + 512)
                w = n1 - n0
                pa = ps_acc.tile([P, 512], F32, space="PSUM", tag="acc")
                t0 = nb2 * 4
                for tt in range(t0, min(SHT, t0 + 4)):
                    col0 = (tt - t0) * P
                    blks = blocks_of_tile[tt]
                    for bi, b in enumerate(blks):
                        it = sb3.tile([P, 1], I32, tag="didx")
                        nc.sync.dma_start(out=it[:],
                                          in_=dep_gidx[b * P:(b + 1) * P, :])
                        gt = sb3.tile([P, EMB], F32, tag="dgather")
                        nc.gpsimd.indirect_dma_start(
                            out=gt[:], out_offset=None, in_=cc_out_t[:, :],
                            in_offset=bass.IndirectOffsetOnAxis(
                                ap=it[:, :1], axis=0))
                        sel = sb3.tile([P, P], F32, tag="dsel")
                        nc.vector.tensor_scalar(
                            out=sel[:], in0=iota_row[:],
                            scalar1=drel_sb[:, b:b + 1], scalar2=None,
                            op0=ALU.is_equal)
                        nc.tensor.matmul(out=pa[:, col0:col0 + P], lhsT=gt[:],
                                         rhs=sel[:], start=(bi == 0),
                                         stop=(bi == len(blks) - 1))
                xl = sb.tile([P, 512], F32, tag="xl")
                nc.sync.dma_start(out=xl[:, :w], in_=x_fm[:, n0:n1])
                zt = sb.tile([P, 512], F32, tag="zt")
                nc.vector.tensor_tensor(out=zt[:, :w], in0=xl[:, :w],
                                        in1=pa[:, :w], op=ALU.add)
                consume(nb2, zt, w)

        # L1 consume: t1 = relu(Wa^T z + ba); h = relu(Wb^T t1 + bb);
        # y2 = Wc^T h  (all per 512-block, nothing persisted)

        def l1_consume(nb2, zt, w):
            n0 = nb2 * 512
            t1 = sb.tile([P, 2 * 512], F32, tag="t1")
            for f in range(2):
                pp = ps.tile([P, 512], F32, space="PSUM", tag="w512")
                nc.tensor.matmul(out=pp[:, :w], lhsT=w_a[:, f * P:(f + 1) * P],
                                 rhs=zt[:, :w], start=True, stop=True)
                nc.scalar.activation(out=t1[:, f * 512:f * 512 + w],
                                     in_=pp[:, :w], func=ACTF.Relu,
                                     bias=v_ba[:, f:f + 1])
            hb = sb.tile([P, 2 * 512], F32, tag="hb")
            for f in range(2):
                pp = ps.tile([P, 512], F32, space="PSUM", tag="w512")
                for k in range(2):
                    nc.tensor.matmul(
                        out=pp[:, :w],
                        lhsT=w_b[:, k * HID + f * P:k * HID + (f + 1) * P],
                        rhs=t1[:, k * 512:k * 512 + w],
                        start=(k == 0), stop=(k == 1))
                nc.scalar.activation(out=hb[:, f * 512:f * 512 + w],
                                     in_=pp[:, :w], func=ACTF.Relu,
                                     bias=v_bb[:, f:f + 1])
            pp = ps.tile([P, 512], F32, space="PSUM", tag="w512")
            for k in range(2):
                nc.tensor.matmul(out=pp[:, :w], lhsT=w_c[:, k * EMB:(k + 1) * EMB],
                                 rhs=hb[:, k * 512:k * 512 + w],
                                 start=(k == 0), stop=(k == 1))
            y2b = sb.tile([P, 512], F32, tag="thb")
            nc.vector.tensor_copy(out=y2b[:, :w], in_=pp[:, :w])
            nc.sync.dma_start(out=y2_fm[:, n0:n0 + w], in_=y2b[:, :w])
            transpose_block([(cc2_in, 0)], y2b, n0, w)

        dep_agg_consume(cc1_out, th_fm, l1_consume)

        # vm L1 + y2vm
        hvm_t = const.tile([P, 2 * NV], F32, tag="vmB")
        zvm = sb.tile([P, NV], F32, tag="zvm")
        nc.vector.tensor_add(out=zvm[:], in0=vmh_t[:], in1=vm_agg1_t[:, :NV])
        t1v = const.tile([P, 2 * NV], F32, tag="vmC")
        for f in range(2):
            for c0, c1, pp in mm_chunks(None, [w_a[:, f * P:(f + 1) * P]],
                                        zvm, [0], NV):
                nc.scalar.activation(out=t1v[:, f * NV + c0:f * NV + c1],
                                     in_=pp[:, :c1 - c0], func=ACTF.Relu,
                                     bias=v_ba[:, f:f + 1])
        for f in range(2):
            for c0, c1, pp in mm_chunks(
                    None,
                    [w_b[:, k * HID + f * P:k * HID + (f + 1) * P] for k in range(2)],
                    t1v, [0, NV], NV):
                nc.scalar.activation(out=hvm_t[:, f * NV + c0:f * NV + c1],
                                     in_=pp[:, :c1 - c0], func=ACTF.Relu,
                                     bias=v_bb[:, f:f + 1])
        y2vm_t = const.tile([P, NV], F32, tag="y2vm")
        for c0, c1, pp in mm_chunks(
                None, [w_c[:, k * EMB:(k + 1) * EMB] for k in range(2)],
                hvm_t, [0, NV], NV):
            nc.vector.tensor_copy(out=y2vm_t[:, c0:c1], in_=pp[:, :c1 - c0])

        scope_out("layer1")
        scope_in("cc2")
        # cc2: y2 row-major + L2 vm partials, AllGather
        if SHP > SH:
            zpad2 = sb.tile([SHP - SH, EMB], F32, tag="zpad")
            nc.gpsimd.memset(zpad2[:], 0.0)
            nc.sync.dma_start(out=cc2_in[SH:SHP, :], in_=zpad2[:])
        compat_agg(cc2_in, cc2_in)
        nc.gpsimd.collective_compute("AllGather", ALU.bypass, replica_groups=RG,
                                     ins=[cc2_in[:]], outs=[cc2_out[:]])
        scope_out("cc2")
        scope_in("layer2")
        vm_agg2_t = vm_agg_reduce(cc2_out, "vagg")


        # L2 consume: t2 = relu(z + bc); ne = Wd^T t2 + bd; outputs per block
        gacc = const.tile([P, 1], F32, tag="gacc")
        nc.gpsimd.memset(gacc[:], 0.0)

        def l2_consume(nb2, zt, w):
            n0 = nb2 * 512
            t2 = sb.tile([P, 512], F32, tag="t2")
            nc.scalar.activation(out=t2[:, :w], in_=zt[:, :w], func=ACTF.Relu,
                                 bias=v_bc[:, 0:1])
            pp = ps.tile([P, 512], F32, space="PSUM", tag="w512")
            nc.tensor.matmul(out=pp[:, :w], lhsT=w_d[:], rhs=t2[:, :w],
                             start=True, stop=True)
            neb = sb.tile([P, 512], F32, tag="thb")
            nc.vector.tensor_tensor(
                out=neb[:, :w], in0=pp[:, :w],
                in1=v_bd[:, 0:1].to_broadcast([P, w]), op=ALU.add)
            gb = sb.tile([P, 1], F32, tag="str1")
            nc.vector.reduce_sum(out=gb[:], in_=neb[:, :w], axis=AX)
            nc.vector.tensor_add(out=gacc[:], in0=gacc[:], in1=gb[:])
            transpose_block([(o_ne, 0), (netask_rm, 0)], neb, n0, w)

        dep_agg_consume(cc2_out, y2_fm, l2_consume)

        # vm L2
        nevm_t = const.tile([P, NV], F32, tag="nevm")
        zv2 = sb.tile([P, NV], F32, tag="zv2")
        nc.vector.tensor_add(out=zv2[:], in0=y2vm_t[:], in1=vm_agg2_t[:, :NV])
        nc.scalar.activation(out=zv2[:], in_=zv2[:], func=ACTF.Relu,
                             bias=v_bc[:, 0:1])
        for c0, c1, pp in mm_chunks(None, [w_d[:]], zv2, [0], NV):
            nc.vector.tensor_tensor(
                out=nevm_t[:, c0:c1], in0=pp[:, :c1 - c0],
                in1=v_bd[:, 0:1].to_broadcast([P, c1 - c0]), op=ALU.add)

        # node embedding outputs (vm part) + graph partials
        if SHP > SH:
            zp3 = sb.tile([SHP - SH, EMB], F32, tag="zpad")
            nc.gpsimd.memset(zp3[:], 0.0)
            nc.sync.dma_start(out=netask_rm[SH:SHP, :], in_=zp3[:])
        for vb in range(math.ceil(NV / 512)):
            n0 = vb * 512
            n1 = min(NV, n0 + 512)
            transpose_block([(o_nevm, 0), (nevm_rm, 0)],
                            nevm_t[:, n0:n1], n0, n1 - n0)
        if NVP > NV:
            zp4 = sb.tile([NVP - NV, EMB], F32, tag="zpad")
            nc.gpsimd.memset(zp4[:], 0.0)
            nc.sync.dma_start(out=nevm_rm[NV:NVP, :], in_=zp4[:])

        gp = sb.tile([P, 2], F32, tag="gp")
        nc.vector.tensor_copy(out=gp[:, 0:1], in_=gacc[:])
        nc.vector.reduce_sum(out=gp[:, 1:2], in_=nevm_t[:, :NV], axis=AX)
        nc.sync.dma_start(out=o_graph[:], in_=gp[:])

        scope_out("layer2")
        scope_in("edges")
        # -------- edge expansions: f-major output [128, B*128], 4 blocks
        # per matmul group; selT via DMA-broadcast + is_equal
        iota_col4_i = const.tile([P, 512], I32, tag="iotc4i")
        nc.gpsimd.iota(iota_col4_i[:], pattern=[[0, 512]], channel_multiplier=1)
        iota_col4 = const.tile([P, 512], F32, tag="iotc4")
        nc.vector.tensor_copy(out=iota_col4[:], in_=iota_col4_i[:])

        def expand_stream(rel_quad_dram, n_blocks, segs, table_dram, out_fm):
            NQ = math.ceil(n_blocks / 4)
            for q in range(NQ):
                wq = min(512, (n_blocks - q * 4) * P)
                bcast = sb.tile([P, 512], F32, tag="bcast")
                nc.sync.dma_start(
                    out=bcast[:],
                    in_=rel_quad_dram[q:q + 1, :].to_broadcast([P, 512]))
                selT4 = sb.tile([P, 512], F32, tag="selT4")
                nc.vector.tensor_tensor(out=selT4[:], in0=iota_col4[:],
                                        in1=bcast[:], op=ALU.is_equal)
                pe = ps.tile([P, 512], F32, space="PSUM", tag="w512")
                for (t, j0, nb2) in segs[q]:
                    ttile = sb3.tile([P, EMB], F32, tag="extab")
                    nc.sync.dma_start(out=ttile[:],
                                      in_=table_dram[t * P:(t + 1) * P, :])
                    nc.tensor.matmul(
                        out=pe[:, j0 * P:(j0 + nb2) * P], lhsT=ttile[:],
                        rhs=selT4[:, j0 * P:(j0 + nb2) * P],
                        start=True, stop=True)
                so = sb.tile([P, 512], F32, tag="exout")
                nc.scalar.activation(out=so[:, :wq], in_=pe[:, :wq],
                                     func=ACTF.Copy)
                nc.sync.dma_start(out=out_fm[:, q * 512:q * 512 + wq],
                                  in_=so[:, :wq])

        expand_stream(left_srel, B_LEFT, meta["left_segs"], netask_rm, o_left)
        expand_stream(rc_drel, B_RC, meta["rc_segs"], nevm_rm, o_rc)
        expand_stream(dep_drel_quad, B_DEP, meta["dep_segs"], netask_rm, o_rd)
        scope_out("edges")

    _split_multiwaits(nc)
    return nc


# ---------------------------------------------------------------- assembly

def _assemble(meta, results, asm):
    c = meta["cfg"]
    NT, NV, E1, E2 = c["NT"], c["NV"], c["E1"], c["E2"]
    SH = c["SH"]
    node = np.zeros((NT + NV, EMB), np.float32)
    for r in range(NCORE):
        node[r * SH:(r + 1) * SH] = results[r]["o_ne"]
    node[NT:] = results[0]["o_nevm"]

    edge = np.zeros((E1 + E2, 2 * EMB), np.float32)
    for r in range(NCORE):
        le = asm["left_eids"][r]
        lk = asm["left_kind"][r]
        lv = np.ascontiguousarray(results[r]["o_left"].T)
        m = le >= 0
        ids = le[m].copy()
        ids[lk[m] == 1] += E1
        edge[ids, :EMB] = lv[m]
        rce = asm["rc_eids"][r]
        m = rce >= 0
        edge[rce[m], EMB:] = np.ascontiguousarray(results[r]["o_rc"].T)[m]
        de = asm["dep_eids"][r]
        m = de >= 0
        edge[de[m] + E1, EMB:] = np.ascontiguousarray(results[r]["o_rd"].T)[m]

    gsum = np.zeros(EMB, np.float32)
    for r in range(NCORE):
        gsum += results[r]["o_graph"][:, 0]
    gsum += results[0]["o_graph"][:, 1]
    graph = (gsum / (NT + NV)).reshape(1, EMB).astype(np.float32)
    return node, edge, graph


# ---------------------------------------------------------------- entry

def kernel(**inputs):
    cfg = CFG_FULL
    meta, per_core, asm = _prep(inputs, cfg)
    nc = _build(meta)
    res = run_bass_kernel_spmd(
        nc, per_core, core_ids=list(range(NCORE)),
        trace=bool(os.environ.get("KERNEL_TRACE")))
    if res.exec_time_ns is not None:
        print(f"HW exec time: {res.exec_time_ns} ns")
    return _assemble(meta, res.results, asm)
